# revision 8
# baseline (speedup 1.0000x reference)
"""Multi-head attention (B=2, N=4096, D=768, H=12) on 8 NeuronCores.

Sharding: core c -> (batch b = c//4, head-group hg = c%4 of 3 heads).
Each core computes Q/K/V projections for its 3 heads from the transposed
input xT (bf16), per-head scores^T = K @ Q^T with keys on partitions,
softmax (exp on ScalarE, denominator via a ones-column folded into the
AV matmul), AV, and the output projection restricted to its heads' rows
of Wo, producing a [4096, 768] fp32 partial. The host sums the four
head-group partials per batch and adds bo (the row-parallel all-reduce
done at unshard time).

PE row-group packing: heads 0+1 share combined [128, N] K^T/Q^T tiles so
their score matmuls run concurrently in disjoint row groups; head 2's
K^T/Q^T rows are duplicated into both halves (via host-duplicated weight
columns) so its score matmuls pair across two query blocks. Half-array
matmuls otherwise leave the PE HAM clock gate at 4/8 (1.2 GHz).
"""

import numpy as np
import ml_dtypes

DIM = 768
NUM_HEADS = 12
HEAD_DIM = 64
SCALE = HEAD_DIM ** -0.5
B = 2
N = 4096
N_CORES = 8
HG = 3               # heads per core
HD3 = HG * HEAD_DIM  # 192
BF16 = ml_dtypes.bfloat16

_cache = {}


def _build_program():
    import concourse.mybir as mybir
    import concourse.tile as tile
    from concourse import bacc

    fp32 = mybir.dt.float32
    bf16 = mybir.dt.bfloat16
    AF = mybir.ActivationFunctionType

    nc = bacc.Bacc("TRN2", target_bir_lowering=False, debug=False,
                   num_devices=N_CORES)

    # wq/wk carry 256 columns: [h0|h1|h2|h2] (head 2 duplicated)
    xt_d = nc.dram_tensor("xt", [DIM, N], bf16, kind="ExternalInput")
    wq_d = nc.dram_tensor("wq", [DIM, 256], bf16, kind="ExternalInput")
    wk_d = nc.dram_tensor("wk", [DIM, 256], bf16, kind="ExternalInput")
    wv_d = nc.dram_tensor("wv", [DIM, HG * 65], bf16, kind="ExternalInput")
    wo_d = nc.dram_tensor("wo", [HD3, DIM], bf16, kind="ExternalInput")
    bq_d = nc.dram_tensor("bq", [1, 256], bf16, kind="ExternalInput")
    bk_d = nc.dram_tensor("bk", [1, 256], bf16, kind="ExternalInput")
    bv_d = nc.dram_tensor("bv", [1, HG * 65], bf16, kind="ExternalInput")
    out_d = nc.dram_tensor("out", [N, DIM], fp32, kind="ExternalOutput")

    KC = DIM // 128      # 6 contraction chunks
    NQB = N // 512       # 8 query blocks of 512
    NKB = N // 128       # 32 key blocks of 128
    V_W = HG * 65        # 195: v columns incl. per-head ones column

    with tile.TileContext(nc) as tc:
        with (
            tc.tile_pool(name="const", bufs=1) as cpool,
            tc.tile_pool(name="big", bufs=1) as bpool,
            tc.tile_pool(name="work", bufs=4) as wpool,
            tc.tile_pool(name="psum", bufs=2, space="PSUM") as ppool,
        ):
            # ---- load inputs; xt split in column halves for a fast start ---
            wq, wk, wv = [], [], []
            for k in range(KC):
                t = cpool.tile([128, 256], bf16, tag=f"wk{k}")
                nc.sync.dma_start(t[:], wk_d.ap()[k * 128:(k + 1) * 128, :])
                wk.append(t)
                t = cpool.tile([128, 256], bf16, tag=f"wq{k}")
                nc.sync.dma_start(t[:], wq_d.ap()[k * 128:(k + 1) * 128, :])
                wq.append(t)
                t = cpool.tile([128, V_W], bf16, tag=f"wv{k}")
                nc.sync.dma_start(t[:], wv_d.ap()[k * 128:(k + 1) * 128, :])
                wv.append(t)
            bq = cpool.tile([1, 256], bf16, tag="bq")
            nc.sync.dma_start(bq[:], bq_d.ap()[:])
            bk = cpool.tile([1, 256], bf16, tag="bk")
            nc.sync.dma_start(bk[:], bk_d.ap()[:])
            bv = cpool.tile([1, V_W], bf16, tag="bv")
            nc.sync.dma_start(bv[:], bv_d.ap()[:])
            ones = cpool.tile([1, 512], bf16, tag="ones")
            nc.gpsimd.memset(ones[:], 1.0)
            xt = []
            for k in range(KC):
                t = cpool.tile([128, N], bf16, tag=f"xt{k}")
                nc.sync.dma_start(t[:, 0:2048],
                                  xt_d.ap()[k * 128:(k + 1) * 128, 0:2048])
                xt.append(t)
            wo = []
            for h in range(HG):
                t = cpool.tile([128, DIM], bf16, tag=f"wo{h}", name=f"wo{h}")
                nc.sync.dma_start(t[0:64, :], wo_d.ap()[h * 64:(h + 1) * 64, :])
                nc.gpsimd.memset(t[64:128, :], 0.0)
                wo.append(t)
            for k in range(KC):
                nc.sync.dma_start(xt[k][:, 2048:4096],
                                  xt_d.ap()[k * 128:(k + 1) * 128, 2048:4096])

            # ---- K^T / Q^T: [h0|h1] combined; [h2|h2] duplicated ----------
            kt01 = bpool.tile([128, N], bf16, tag="kt01")
            qt01 = bpool.tile([128, N], bf16, tag="qt01")
            kt2 = bpool.tile([128, N], bf16, tag="kt2")
            qt2 = bpool.tile([128, N], bf16, tag="qt2")
            for nb in range(NQB):
                csl = slice(nb * 512, (nb + 1) * 512)
                for dst01, dst2, w, bias in ((kt01, kt2, wk, bk),
                                             (qt01, qt2, wq, bq)):
                    for m, dst in ((0, dst01), (1, dst2)):
                        msl = slice(m * 128, (m + 1) * 128)
                        ps = ppool.tile([128, 512], fp32, tag="proj")
                        for k in range(KC):
                            nc.tensor.matmul(ps[:], w[k][:, msl],
                                             xt[k][:, csl],
                                             start=(k == 0), stop=False)
                        nc.tensor.matmul(ps[:], bias[:, msl], ones[:],
                                         start=False, stop=True)
                        nc.vector.tensor_copy(dst[:, csl], ps[:])

            # ---- V: [128, 195] per seq block; col h*65+64 is the ones col --
            v_sb = bpool.tile([128, NKB * V_W], bf16, tag="v")
            for s in range(NKB):
                ssl = slice(s * 128, (s + 1) * 128)
                ps = ppool.tile([128, V_W], fp32, tag="proj")
                for k in range(KC):
                    nc.tensor.matmul(ps[:], xt[k][:, ssl], wv[k][:],
                                     start=(k == 0), stop=False)
                nc.tensor.matmul(ps[:], ones[:, 0:128], bv[:],
                                 start=False, stop=True)
                nc.vector.tensor_copy(v_sb[:, s * V_W:(s + 1) * V_W], ps[:])

            # ---- attention; attn tiles padded to 128 rows (zeros above) ----
            attn = []
            for h in range(HG):
                t = bpool.tile([128, N], bf16, tag=f"attn{h}", name=f"attn{h}")
                nc.gpsimd.memset(t[64:128, :], 0.0)
                attn.append(t)

            def vsl(kb, h):
                return slice(kb * V_W + h * 65, kb * V_W + h * 65 + 65)

            def normalize(av, h, qsl):
                # decouple from PSUM quickly, then recip/bcast/mul from SBUF
                av_sb = wpool.tile([65, 512], fp32, tag="av_sb")
                nc.vector.tensor_copy(av_sb[:], av[:])
                r_row = wpool.tile([1, 512], fp32, tag="r_row")
                nc.vector.reciprocal(r_row[:], av_sb[64:65, :])
                r_bc = wpool.tile([64, 512], fp32, tag="r_bc")
                nc.gpsimd.partition_broadcast(r_bc[:], r_row[:])
                nc.gpsimd.tensor_mul(attn[h][0:64, qsl], av_sb[0:64, :], r_bc[:])

            def heads01(qsl):
                av0 = ppool.tile([65, 512], fp32, tag="av")
                av1 = ppool.tile([65, 512], fp32, tag="av")
                for kb in range(NKB):
                    ksl = slice(kb * 128, (kb + 1) * 128)
                    sc = ppool.tile([128, 1024], fp32, tag="scores")
                    nc.tensor.matmul(sc[:, 0:512], kt01[0:64, ksl],
                                     qt01[0:64, qsl], start=True, stop=True)
                    nc.tensor.matmul(sc[:, 512:1024], kt01[64:128, ksl],
                                     qt01[64:128, qsl], start=True, stop=True)
                    probs = wpool.tile([128, 1024], bf16, tag="probs")
                    nc.scalar.activation(probs[:], sc[:], AF.Exp)
                    nc.tensor.matmul(av0[:], v_sb[:, vsl(kb, 0)],
                                     probs[:, 0:512],
                                     start=(kb == 0), stop=(kb == NKB - 1))
                    nc.tensor.matmul(av1[:], v_sb[:, vsl(kb, 1)],
                                     probs[:, 512:1024],
                                     start=(kb == 0), stop=(kb == NKB - 1))
                return av0, av1

            for qp in range(NQB // 2):
                qsl0 = slice((2 * qp) * 512, (2 * qp + 1) * 512)
                qsl1 = slice((2 * qp + 1) * 512, (2 * qp + 2) * 512)
                av0, av1 = heads01(qsl0)
                normalize(av0, 0, qsl0)
                normalize(av1, 1, qsl0)
                av0b, av1b = heads01(qsl1)
                normalize(av0b, 0, qsl1)
                normalize(av1b, 1, qsl1)
                # head 2: both query blocks concurrently (duplicated rows)
                av2 = ppool.tile([65, 512], fp32, tag="av")
                av2b = ppool.tile([65, 512], fp32, tag="av")
                for kb in range(NKB):
                    ksl = slice(kb * 128, (kb + 1) * 128)
                    sc = ppool.tile([128, 1024], fp32, tag="scores")
                    nc.tensor.matmul(sc[:, 0:512], kt2[0:64, ksl],
                                     qt2[0:64, qsl0], start=True, stop=True)
                    nc.tensor.matmul(sc[:, 512:1024], kt2[64:128, ksl],
                                     qt2[64:128, qsl1], start=True, stop=True)
                    probs = wpool.tile([128, 1024], bf16, tag="probs")
                    nc.scalar.activation(probs[:], sc[:], AF.Exp)
                    nc.tensor.matmul(av2[:], v_sb[:, vsl(kb, 2)],
                                     probs[:, 0:512],
                                     start=(kb == 0), stop=(kb == NKB - 1))
                    nc.tensor.matmul(av2b[:], v_sb[:, vsl(kb, 2)],
                                     probs[:, 512:1024],
                                     start=(kb == 0), stop=(kb == NKB - 1))
                normalize(av2, 2, qsl0)
                normalize(av2b, 2, qsl1)

                # ---- output projection for this query-block pair ----------
                for s in range(8 * qp, 8 * (qp + 1)):
                    ssl = slice(s * 128, (s + 1) * 128)
                    ob = wpool.tile([128, DIM], fp32, tag="out_sb")
                    for n2 in range(2):
                        nsl = slice(n2 * 384, (n2 + 1) * 384)
                        ps = ppool.tile([128, 384], fp32, tag="proj")
                        for h in range(HG):
                            nc.tensor.matmul(ps[:], attn[h][:, ssl],
                                             wo[h][:, nsl],
                                             start=(h == 0), stop=(h == HG - 1))
                        nc.vector.tensor_copy(ob[:, nsl], ps[:])
                    nc.sync.dma_start(out_d.ap()[ssl, :], ob[:])

    nc.compile()
    return nc


def _get_program():
    if "nc" not in _cache:
        _cache["nc"] = _build_program()
    return _cache["nc"]


def _make_in_maps(x, Wq, bq, Wk, bk, Wv, bv, Wo):
    in_maps = []
    for c in range(N_CORES):
        b, hg = divmod(c, 4)
        sl = slice(HD3 * hg, HD3 * (hg + 1))
        # [h0|h1|h2|h2]: head 2 duplicated into both PE row-group halves
        def ext(W_sl):
            return np.concatenate([W_sl, W_sl[..., 128:192]], axis=-1)
        wv_ext = np.zeros((DIM, HG * 65), np.float32)
        bv_ext = np.zeros((1, HG * 65), np.float32)
        for h in range(HG):
            wv_ext[:, h * 65:h * 65 + 64] = Wv[:, HD3 * hg + h * 64:HD3 * hg + (h + 1) * 64]
            bv_ext[0, h * 65:h * 65 + 64] = bv[HD3 * hg + h * 64:HD3 * hg + (h + 1) * 64]
            bv_ext[0, h * 65 + 64] = 1.0
        in_maps.append({
            "xt": np.ascontiguousarray(x[b].T).astype(BF16),
            "wq": ext(Wq[:, sl] * SCALE).astype(BF16),
            "wk": ext(Wk[:, sl]).astype(BF16),
            "wv": wv_ext.astype(BF16),
            "wo": Wo[sl, :].astype(BF16),
            "bq": ext(bq[None, sl] * SCALE).astype(BF16),
            "bk": ext(bk[None, sl]).astype(BF16),
            "bv": bv_ext.astype(BF16),
        })
    return in_maps


def kernel(x, Wq, bq, Wk, bk, Wv, bv, Wo, bo):
    from concourse import bass_utils

    x = np.asarray(x, np.float32)
    Wq = np.asarray(Wq, np.float32); bq = np.asarray(bq, np.float32)
    Wk = np.asarray(Wk, np.float32); bk = np.asarray(bk, np.float32)
    Wv = np.asarray(Wv, np.float32); bv = np.asarray(bv, np.float32)
    Wo = np.asarray(Wo, np.float32); bo = np.asarray(bo, np.float32)

    nc = _get_program()
    in_maps = _make_in_maps(x, Wq, bq, Wk, bk, Wv, bv, Wo)
    _cache["in_maps"] = in_maps
    res = bass_utils.run_bass_kernel_spmd(nc, in_maps, core_ids=list(range(N_CORES)))
    _cache["last_results"] = res

    out = np.zeros((B, N, DIM), np.float32)
    for c in range(N_CORES):
        out[c // 4] += res.results[c]["out"]
    out += bo[None, None, :]
    return out


# revision 9
# speedup vs baseline: 1.1971x; 1.1971x over previous
"""Multi-head attention (B=2, N=4096, D=768, H=12) on 8 NeuronCores.

Sharding: core c -> (batch b = c//4, head-group hg = c%4 of 3 heads).
Each core computes Q/K/V projections for its 3 heads from the transposed
input xT (bf16), per-head scores^T = K @ Q^T with keys on partitions,
softmax (exp on ScalarE, denominator via a ones-column folded into the
AV matmul), AV, and the output projection restricted to its heads' rows
of Wo, producing a [4096, 768] fp32 partial. The host sums the four
head-group partials per batch and adds bo (the row-parallel all-reduce
done at unshard time).

PE row-group packing: heads 0+1 share combined [128, N] K^T/Q^T tiles so
their score matmuls run concurrently in disjoint row groups; head 2's
K^T/Q^T rows are duplicated into both halves (via host-duplicated weight
columns) so its score matmuls pair across two query blocks. Half-array
matmuls otherwise leave the PE HAM clock gate at 4/8 (1.2 GHz).
"""

import numpy as np
import ml_dtypes

DIM = 768
NUM_HEADS = 12
HEAD_DIM = 64
SCALE = HEAD_DIM ** -0.5
B = 2
N = 4096
N_CORES = 8
HG = 3               # heads per core
HD3 = HG * HEAD_DIM  # 192
BF16 = ml_dtypes.bfloat16

_cache = {}


def _build_program():
    import concourse.mybir as mybir
    import concourse.tile as tile
    from concourse import bacc

    fp32 = mybir.dt.float32
    bf16 = mybir.dt.bfloat16
    AF = mybir.ActivationFunctionType

    nc = bacc.Bacc("TRN2", target_bir_lowering=False, debug=False,
                   num_devices=N_CORES)

    # wq/wk carry 256 columns: [h0|h1|h2|h2] (head 2 duplicated)
    xt_d = nc.dram_tensor("xt", [DIM, N], bf16, kind="ExternalInput")
    wq_d = nc.dram_tensor("wq", [DIM, 256], bf16, kind="ExternalInput")
    wk_d = nc.dram_tensor("wk", [DIM, 256], bf16, kind="ExternalInput")
    wv_d = nc.dram_tensor("wv", [DIM, HG * 65], bf16, kind="ExternalInput")
    wo_d = nc.dram_tensor("wo", [HD3, DIM], bf16, kind="ExternalInput")
    bq_d = nc.dram_tensor("bq", [1, 256], bf16, kind="ExternalInput")
    bk_d = nc.dram_tensor("bk", [1, 256], bf16, kind="ExternalInput")
    bv_d = nc.dram_tensor("bv", [1, HG * 65], bf16, kind="ExternalInput")
    out_d = nc.dram_tensor("out", [N, DIM], fp32, kind="ExternalOutput")

    KC = DIM // 128      # 6 contraction chunks
    NQB = N // 512       # 8 query blocks of 512
    NKB = N // 128       # 32 key blocks of 128
    V_W = HG * 65        # 195: v columns incl. per-head ones column

    with tile.TileContext(nc) as tc:
        with (
            tc.tile_pool(name="const", bufs=1) as cpool,
            tc.tile_pool(name="big", bufs=1) as bpool,
            tc.tile_pool(name="work", bufs=4) as wpool,
            tc.tile_pool(name="psum", bufs=2, space="PSUM") as ppool,
        ):
            # ---- load inputs; xt split in column halves for a fast start ---
            wq, wk, wv = [], [], []
            for k in range(KC):
                t = cpool.tile([128, 256], bf16, tag=f"wk{k}")
                nc.sync.dma_start(t[:], wk_d.ap()[k * 128:(k + 1) * 128, :])
                wk.append(t)
                t = cpool.tile([128, 256], bf16, tag=f"wq{k}")
                nc.sync.dma_start(t[:], wq_d.ap()[k * 128:(k + 1) * 128, :])
                wq.append(t)
                t = cpool.tile([128, V_W], bf16, tag=f"wv{k}")
                nc.sync.dma_start(t[:], wv_d.ap()[k * 128:(k + 1) * 128, :])
                wv.append(t)
            bq = cpool.tile([1, 256], bf16, tag="bq")
            nc.sync.dma_start(bq[:], bq_d.ap()[:])
            bk = cpool.tile([1, 256], bf16, tag="bk")
            nc.sync.dma_start(bk[:], bk_d.ap()[:])
            bv = cpool.tile([1, V_W], bf16, tag="bv")
            nc.sync.dma_start(bv[:], bv_d.ap()[:])
            ones = cpool.tile([1, 512], bf16, tag="ones")
            nc.gpsimd.memset(ones[:], 1.0)
            xt = []
            for k in range(KC):
                t = cpool.tile([128, N], bf16, tag=f"xt{k}")
                nc.sync.dma_start(t[:, 0:2048],
                                  xt_d.ap()[k * 128:(k + 1) * 128, 0:2048])
                xt.append(t)
            wo = []
            for h in range(HG):
                t = cpool.tile([128, DIM], bf16, tag=f"wo{h}", name=f"wo{h}")
                nc.sync.dma_start(t[0:64, :], wo_d.ap()[h * 64:(h + 1) * 64, :])
                nc.gpsimd.memset(t[64:128, :], 0.0)
                wo.append(t)
            for k in range(KC):
                nc.sync.dma_start(xt[k][:, 2048:4096],
                                  xt_d.ap()[k * 128:(k + 1) * 128, 2048:4096])

            # ---- K^T / Q^T: [h0|h1] combined; [h2|h2] duplicated ----------
            kt01 = bpool.tile([128, N], bf16, tag="kt01")
            qt01 = bpool.tile([128, N], bf16, tag="qt01")
            kt2 = bpool.tile([128, N], bf16, tag="kt2")
            qt2 = bpool.tile([128, N], bf16, tag="qt2")
            for nb in range(NQB):
                csl = slice(nb * 512, (nb + 1) * 512)
                for dst01, dst2, w, bias in ((kt01, kt2, wk, bk),
                                             (qt01, qt2, wq, bq)):
                    for m, dst in ((0, dst01), (1, dst2)):
                        msl = slice(m * 128, (m + 1) * 128)
                        ps = ppool.tile([128, 512], fp32, tag="proj")
                        for k in range(KC):
                            nc.tensor.matmul(ps[:], w[k][:, msl],
                                             xt[k][:, csl],
                                             start=(k == 0), stop=False)
                        nc.tensor.matmul(ps[:], bias[:, msl], ones[:],
                                         start=False, stop=True)
                        nc.vector.tensor_copy(dst[:, csl], ps[:])

            # ---- V: [128, 195] per seq block; col h*65+64 is the ones col --
            v_sb = bpool.tile([128, NKB * V_W], bf16, tag="v")
            for s in range(NKB):
                ssl = slice(s * 128, (s + 1) * 128)
                ps = ppool.tile([128, V_W], fp32, tag="proj")
                for k in range(KC):
                    nc.tensor.matmul(ps[:], xt[k][:, ssl], wv[k][:],
                                     start=(k == 0), stop=False)
                nc.tensor.matmul(ps[:], ones[:, 0:128], bv[:],
                                 start=False, stop=True)
                nc.vector.tensor_copy(v_sb[:, s * V_W:(s + 1) * V_W], ps[:])

            # ---- attention; attn tiles padded to 128 rows (zeros above) ----
            attn = []
            for h in range(HG):
                t = bpool.tile([128, N], bf16, tag=f"attn{h}", name=f"attn{h}")
                nc.gpsimd.memset(t[64:128, :], 0.0)
                attn.append(t)

            def vsl(kb, h):
                return slice(kb * V_W + h * 65, kb * V_W + h * 65 + 65)

            def normalize(av, h, qsl):
                # decouple from PSUM quickly, then recip/bcast/mul from SBUF
                av_sb = wpool.tile([65, 512], fp32, tag="av_sb")
                nc.vector.tensor_copy(av_sb[:], av[:])
                r_row = wpool.tile([1, 512], fp32, tag="r_row")
                nc.vector.reciprocal(r_row[:], av_sb[64:65, :])
                r_bc = wpool.tile([64, 512], fp32, tag="r_bc")
                nc.gpsimd.partition_broadcast(r_bc[:], r_row[:])
                nc.vector.tensor_mul(attn[h][0:64, qsl], av_sb[0:64, :], r_bc[:])

            def heads01(qsl):
                av0 = ppool.tile([65, 512], fp32, tag="av")
                av1 = ppool.tile([65, 512], fp32, tag="av")
                for kb in range(NKB):
                    ksl = slice(kb * 128, (kb + 1) * 128)
                    sc = ppool.tile([128, 1024], fp32, tag="scores")
                    nc.tensor.matmul(sc[:, 0:512], kt01[0:64, ksl],
                                     qt01[0:64, qsl], start=True, stop=True)
                    nc.tensor.matmul(sc[:, 512:1024], kt01[64:128, ksl],
                                     qt01[64:128, qsl], start=True, stop=True)
                    probs = wpool.tile([128, 1024], bf16, tag="probs")
                    nc.scalar.activation(probs[:], sc[:], AF.Exp)
                    nc.tensor.matmul(av0[:], v_sb[:, vsl(kb, 0)],
                                     probs[:, 0:512],
                                     start=(kb == 0), stop=(kb == NKB - 1))
                    nc.tensor.matmul(av1[:], v_sb[:, vsl(kb, 1)],
                                     probs[:, 512:1024],
                                     start=(kb == 0), stop=(kb == NKB - 1))
                return av0, av1

            for qp in range(NQB // 2):
                qsl0 = slice((2 * qp) * 512, (2 * qp + 1) * 512)
                qsl1 = slice((2 * qp + 1) * 512, (2 * qp + 2) * 512)
                av0, av1 = heads01(qsl0)
                normalize(av0, 0, qsl0)
                normalize(av1, 1, qsl0)
                av0b, av1b = heads01(qsl1)
                normalize(av0b, 0, qsl1)
                normalize(av1b, 1, qsl1)
                # head 2: both query blocks concurrently (duplicated rows)
                av2 = ppool.tile([65, 512], fp32, tag="av")
                av2b = ppool.tile([65, 512], fp32, tag="av")
                for kb in range(NKB):
                    ksl = slice(kb * 128, (kb + 1) * 128)
                    sc = ppool.tile([128, 1024], fp32, tag="scores")
                    nc.tensor.matmul(sc[:, 0:512], kt2[0:64, ksl],
                                     qt2[0:64, qsl0], start=True, stop=True)
                    nc.tensor.matmul(sc[:, 512:1024], kt2[64:128, ksl],
                                     qt2[64:128, qsl1], start=True, stop=True)
                    probs = wpool.tile([128, 1024], bf16, tag="probs")
                    nc.scalar.activation(probs[:], sc[:], AF.Exp)
                    nc.tensor.matmul(av2[:], v_sb[:, vsl(kb, 2)],
                                     probs[:, 0:512],
                                     start=(kb == 0), stop=(kb == NKB - 1))
                    nc.tensor.matmul(av2b[:], v_sb[:, vsl(kb, 2)],
                                     probs[:, 512:1024],
                                     start=(kb == 0), stop=(kb == NKB - 1))
                normalize(av2, 2, qsl0)
                normalize(av2b, 2, qsl1)

                # ---- output projection for this query-block pair ----------
                for s in range(8 * qp, 8 * (qp + 1)):
                    ssl = slice(s * 128, (s + 1) * 128)
                    ob = wpool.tile([128, DIM], fp32, tag="out_sb")
                    for n2 in range(2):
                        nsl = slice(n2 * 384, (n2 + 1) * 384)
                        ps = ppool.tile([128, 384], fp32, tag="proj")
                        for h in range(HG):
                            nc.tensor.matmul(ps[:], attn[h][:, ssl],
                                             wo[h][:, nsl],
                                             start=(h == 0), stop=(h == HG - 1))
                        nc.vector.tensor_copy(ob[:, nsl], ps[:])
                    nc.sync.dma_start(out_d.ap()[ssl, :], ob[:])

    nc.compile()
    return nc


def _get_program():
    if "nc" not in _cache:
        _cache["nc"] = _build_program()
    return _cache["nc"]


def _make_in_maps(x, Wq, bq, Wk, bk, Wv, bv, Wo):
    in_maps = []
    for c in range(N_CORES):
        b, hg = divmod(c, 4)
        sl = slice(HD3 * hg, HD3 * (hg + 1))
        # [h0|h1|h2|h2]: head 2 duplicated into both PE row-group halves
        def ext(W_sl):
            return np.concatenate([W_sl, W_sl[..., 128:192]], axis=-1)
        wv_ext = np.zeros((DIM, HG * 65), np.float32)
        bv_ext = np.zeros((1, HG * 65), np.float32)
        for h in range(HG):
            wv_ext[:, h * 65:h * 65 + 64] = Wv[:, HD3 * hg + h * 64:HD3 * hg + (h + 1) * 64]
            bv_ext[0, h * 65:h * 65 + 64] = bv[HD3 * hg + h * 64:HD3 * hg + (h + 1) * 64]
            bv_ext[0, h * 65 + 64] = 1.0
        in_maps.append({
            "xt": np.ascontiguousarray(x[b].T).astype(BF16),
            "wq": ext(Wq[:, sl] * SCALE).astype(BF16),
            "wk": ext(Wk[:, sl]).astype(BF16),
            "wv": wv_ext.astype(BF16),
            "wo": Wo[sl, :].astype(BF16),
            "bq": ext(bq[None, sl] * SCALE).astype(BF16),
            "bk": ext(bk[None, sl]).astype(BF16),
            "bv": bv_ext.astype(BF16),
        })
    return in_maps


def kernel(x, Wq, bq, Wk, bk, Wv, bv, Wo, bo):
    from concourse import bass_utils

    x = np.asarray(x, np.float32)
    Wq = np.asarray(Wq, np.float32); bq = np.asarray(bq, np.float32)
    Wk = np.asarray(Wk, np.float32); bk = np.asarray(bk, np.float32)
    Wv = np.asarray(Wv, np.float32); bv = np.asarray(bv, np.float32)
    Wo = np.asarray(Wo, np.float32); bo = np.asarray(bo, np.float32)

    nc = _get_program()
    in_maps = _make_in_maps(x, Wq, bq, Wk, bk, Wv, bv, Wo)
    _cache["in_maps"] = in_maps
    res = bass_utils.run_bass_kernel_spmd(nc, in_maps, core_ids=list(range(N_CORES)))
    _cache["last_results"] = res

    out = np.zeros((B, N, DIM), np.float32)
    for c in range(N_CORES):
        out[c // 4] += res.results[c]["out"]
    out += bo[None, None, :]
    return out


# revision 11
# speedup vs baseline: 1.1992x; 1.0017x over previous
"""Multi-head attention (B=2, N=4096, D=768, H=12) on 8 NeuronCores.

Sharding: core c -> (batch b = c//4, head-group hg = c%4 of 3 heads).
Each core computes Q/K/V projections for its 3 heads from the transposed
input xT (bf16), per-head scores^T = K @ Q^T with keys on partitions,
softmax (exp on ScalarE, denominator via a ones-column folded into the
AV matmul), AV, and the output projection restricted to its heads' rows
of Wo, producing a [4096, 768] fp32 partial. The host sums the four
head-group partials per batch and adds bo (the row-parallel all-reduce
done at unshard time).

PE row-group packing: heads 0+1 share combined [128, N] K^T/Q^T tiles so
their score matmuls run concurrently in disjoint row groups; head 2's
K^T/Q^T rows are duplicated into both halves (via host-duplicated weight
columns) so its score matmuls pair across two query blocks. Half-array
matmuls otherwise leave the PE HAM clock gate at 4/8 (1.2 GHz).
"""

import numpy as np
import ml_dtypes

DIM = 768
NUM_HEADS = 12
HEAD_DIM = 64
SCALE = HEAD_DIM ** -0.5
B = 2
N = 4096
N_CORES = 8
HG = 3               # heads per core
HD3 = HG * HEAD_DIM  # 192
BF16 = ml_dtypes.bfloat16

_cache = {}


def _build_program():
    import concourse.mybir as mybir
    import concourse.tile as tile
    from concourse import bacc

    fp32 = mybir.dt.float32
    bf16 = mybir.dt.bfloat16
    AF = mybir.ActivationFunctionType

    nc = bacc.Bacc("TRN2", target_bir_lowering=False, debug=False,
                   num_devices=N_CORES)

    # wq/wk carry 256 columns: [h0|h1|h2|h2] (head 2 duplicated)
    xt_d = nc.dram_tensor("xt", [DIM, N], bf16, kind="ExternalInput")
    wq_d = nc.dram_tensor("wq", [DIM, 256], bf16, kind="ExternalInput")
    wk_d = nc.dram_tensor("wk", [DIM, 256], bf16, kind="ExternalInput")
    wv_d = nc.dram_tensor("wv", [DIM, HG * 65], bf16, kind="ExternalInput")
    wo_d = nc.dram_tensor("wo", [HD3, DIM], bf16, kind="ExternalInput")
    bq_d = nc.dram_tensor("bq", [1, 256], bf16, kind="ExternalInput")
    bk_d = nc.dram_tensor("bk", [1, 256], bf16, kind="ExternalInput")
    bv_d = nc.dram_tensor("bv", [1, HG * 65], bf16, kind="ExternalInput")
    out_d = nc.dram_tensor("out", [N, DIM], fp32, kind="ExternalOutput")

    KC = DIM // 128      # 6 contraction chunks
    NQB = N // 512       # 8 query blocks of 512
    NKB = N // 128       # 32 key blocks of 128
    V_W = HG * 65        # 195: v columns incl. per-head ones column

    with tile.TileContext(nc) as tc:
        with (
            tc.tile_pool(name="const", bufs=1) as cpool,
            tc.tile_pool(name="big", bufs=1) as bpool,
            tc.tile_pool(name="work", bufs=4) as wpool,
            tc.tile_pool(name="psum", bufs=2, space="PSUM") as ppool,
        ):
            # ---- load inputs; xt split in column halves for a fast start ---
            wq, wk, wv = [], [], []
            for k in range(KC):
                t = cpool.tile([128, 256], bf16, tag=f"wk{k}")
                nc.sync.dma_start(t[:], wk_d.ap()[k * 128:(k + 1) * 128, :])
                wk.append(t)
                t = cpool.tile([128, 256], bf16, tag=f"wq{k}")
                nc.sync.dma_start(t[:], wq_d.ap()[k * 128:(k + 1) * 128, :])
                wq.append(t)
                t = cpool.tile([128, V_W], bf16, tag=f"wv{k}")
                nc.sync.dma_start(t[:], wv_d.ap()[k * 128:(k + 1) * 128, :])
                wv.append(t)
            bq = cpool.tile([1, 256], bf16, tag="bq")
            nc.sync.dma_start(bq[:], bq_d.ap()[:])
            bk = cpool.tile([1, 256], bf16, tag="bk")
            nc.sync.dma_start(bk[:], bk_d.ap()[:])
            bv = cpool.tile([1, V_W], bf16, tag="bv")
            nc.sync.dma_start(bv[:], bv_d.ap()[:])
            ones = cpool.tile([1, 512], bf16, tag="ones")
            nc.gpsimd.memset(ones[:], 1.0)
            xt = []
            for k in range(KC):
                t = cpool.tile([128, N], bf16, tag=f"xt{k}")
                nc.sync.dma_start(t[:, 0:2048],
                                  xt_d.ap()[k * 128:(k + 1) * 128, 0:2048])
                xt.append(t)
            wo = []
            for h in range(HG):
                t = cpool.tile([128, DIM], bf16, tag=f"wo{h}", name=f"wo{h}")
                nc.sync.dma_start(t[0:64, :], wo_d.ap()[h * 64:(h + 1) * 64, :])
                nc.gpsimd.memset(t[64:128, :], 0.0)
                wo.append(t)
            for k in range(KC):
                nc.sync.dma_start(xt[k][:, 2048:4096],
                                  xt_d.ap()[k * 128:(k + 1) * 128, 2048:4096])

            # ---- K^T / Q^T: [h0|h1] combined; [h2|h2] duplicated ----------
            kt01 = bpool.tile([128, N], bf16, tag="kt01")
            qt01 = bpool.tile([128, N], bf16, tag="qt01")
            kt2 = bpool.tile([128, N], bf16, tag="kt2")
            qt2 = bpool.tile([128, N], bf16, tag="qt2")
            for nb in range(NQB):
                csl = slice(nb * 512, (nb + 1) * 512)
                for dst01, dst2, w, bias in ((kt01, kt2, wk, bk),
                                             (qt01, qt2, wq, bq)):
                    for m, dst in ((0, dst01), (1, dst2)):
                        msl = slice(m * 128, (m + 1) * 128)
                        ps = ppool.tile([128, 512], fp32, tag="proj")
                        for k in range(KC):
                            nc.tensor.matmul(ps[:], w[k][:, msl],
                                             xt[k][:, csl],
                                             start=(k == 0), stop=False)
                        nc.tensor.matmul(ps[:], bias[:, msl], ones[:],
                                         start=False, stop=True)
                        nc.vector.tensor_copy(dst[:, csl], ps[:])

            # ---- V: [128, 195] per seq block; col h*65+64 is the ones col --
            v_sb = bpool.tile([128, NKB * V_W], bf16, tag="v")
            for s in range(NKB):
                ssl = slice(s * 128, (s + 1) * 128)
                ps = ppool.tile([128, V_W], fp32, tag="proj")
                for k in range(KC):
                    nc.tensor.matmul(ps[:], xt[k][:, ssl], wv[k][:],
                                     start=(k == 0), stop=False)
                nc.tensor.matmul(ps[:], ones[:, 0:128], bv[:],
                                 start=False, stop=True)
                nc.vector.tensor_copy(v_sb[:, s * V_W:(s + 1) * V_W], ps[:])

            # ---- attention; attn tiles padded to 128 rows (zeros above) ----
            attn = []
            for h in range(HG):
                t = bpool.tile([128, N], bf16, tag=f"attn{h}", name=f"attn{h}")
                nc.gpsimd.memset(t[64:128, :], 0.0)
                attn.append(t)

            def vsl(kb, h):
                return slice(kb * V_W + h * 65, kb * V_W + h * 65 + 65)

            def unload(av):
                # free the PSUM slot quickly; DVE FIFO stays short here
                av_sb = wpool.tile([65, 512], fp32, tag="av_sb")
                nc.vector.tensor_copy(av_sb[:], av[:])
                return av_sb

            def finish_norm(av_sb, h, qsl):
                r_row = wpool.tile([1, 512], fp32, tag="r_row")
                nc.vector.reciprocal(r_row[:], av_sb[64:65, :])
                r_bc = wpool.tile([64, 512], fp32, tag="r_bc")
                nc.gpsimd.partition_broadcast(r_bc[:], r_row[:])
                nc.vector.tensor_mul(attn[h][0:64, qsl], av_sb[0:64, :], r_bc[:])

            def heads01(qsl):
                av0 = ppool.tile([65, 512], fp32, tag="av")
                av1 = ppool.tile([65, 512], fp32, tag="av")
                for kb in range(NKB):
                    ksl = slice(kb * 128, (kb + 1) * 128)
                    sc = ppool.tile([128, 1024], fp32, tag="scores")
                    nc.tensor.matmul(sc[:, 0:512], kt01[0:64, ksl],
                                     qt01[0:64, qsl], start=True, stop=True)
                    nc.tensor.matmul(sc[:, 512:1024], kt01[64:128, ksl],
                                     qt01[64:128, qsl], start=True, stop=True)
                    probs = wpool.tile([128, 1024], bf16, tag="probs")
                    nc.scalar.activation(probs[:], sc[:], AF.Exp)
                    nc.tensor.matmul(av0[:], v_sb[:, vsl(kb, 0)],
                                     probs[:, 0:512],
                                     start=(kb == 0), stop=(kb == NKB - 1))
                    nc.tensor.matmul(av1[:], v_sb[:, vsl(kb, 1)],
                                     probs[:, 512:1024],
                                     start=(kb == 0), stop=(kb == NKB - 1))
                return av0, av1

            for qp in range(NQB // 2):
                qsl0 = slice((2 * qp) * 512, (2 * qp + 1) * 512)
                qsl1 = slice((2 * qp + 1) * 512, (2 * qp + 2) * 512)
                av0, av1 = heads01(qsl0)
                sb0 = unload(av0)
                sb1 = unload(av1)
                av0b, av1b = heads01(qsl1)
                finish_norm(sb0, 0, qsl0)
                finish_norm(sb1, 1, qsl0)
                sb0b = unload(av0b)
                sb1b = unload(av1b)
                # head 2: both query blocks concurrently (duplicated rows)
                av2 = ppool.tile([65, 512], fp32, tag="av")
                av2b = ppool.tile([65, 512], fp32, tag="av")
                for kb in range(NKB):
                    ksl = slice(kb * 128, (kb + 1) * 128)
                    sc = ppool.tile([128, 1024], fp32, tag="scores")
                    nc.tensor.matmul(sc[:, 0:512], kt2[0:64, ksl],
                                     qt2[0:64, qsl0], start=True, stop=True)
                    nc.tensor.matmul(sc[:, 512:1024], kt2[64:128, ksl],
                                     qt2[64:128, qsl1], start=True, stop=True)
                    probs = wpool.tile([128, 1024], bf16, tag="probs")
                    nc.scalar.activation(probs[:], sc[:], AF.Exp)
                    nc.tensor.matmul(av2[:], v_sb[:, vsl(kb, 2)],
                                     probs[:, 0:512],
                                     start=(kb == 0), stop=(kb == NKB - 1))
                    nc.tensor.matmul(av2b[:], v_sb[:, vsl(kb, 2)],
                                     probs[:, 512:1024],
                                     start=(kb == 0), stop=(kb == NKB - 1))
                finish_norm(sb0b, 0, qsl1)
                finish_norm(sb1b, 1, qsl1)
                sb2 = unload(av2)
                sb2b = unload(av2b)
                finish_norm(sb2, 2, qsl0)
                finish_norm(sb2b, 2, qsl1)

                # ---- output projection for this query-block pair ----------
                for s in range(8 * qp, 8 * (qp + 1)):
                    ssl = slice(s * 128, (s + 1) * 128)
                    ob = wpool.tile([128, DIM], fp32, tag="out_sb")
                    for n2 in range(2):
                        nsl = slice(n2 * 384, (n2 + 1) * 384)
                        ps = ppool.tile([128, 384], fp32, tag="proj")
                        for h in range(HG):
                            nc.tensor.matmul(ps[:], attn[h][:, ssl],
                                             wo[h][:, nsl],
                                             start=(h == 0), stop=(h == HG - 1))
                        nc.vector.tensor_copy(ob[:, nsl], ps[:])
                    nc.sync.dma_start(out_d.ap()[ssl, :], ob[:])

    nc.compile()
    return nc


def _get_program():
    if "nc" not in _cache:
        _cache["nc"] = _build_program()
    return _cache["nc"]


def _make_in_maps(x, Wq, bq, Wk, bk, Wv, bv, Wo):
    in_maps = []
    for c in range(N_CORES):
        b, hg = divmod(c, 4)
        sl = slice(HD3 * hg, HD3 * (hg + 1))
        # [h0|h1|h2|h2]: head 2 duplicated into both PE row-group halves
        def ext(W_sl):
            return np.concatenate([W_sl, W_sl[..., 128:192]], axis=-1)
        wv_ext = np.zeros((DIM, HG * 65), np.float32)
        bv_ext = np.zeros((1, HG * 65), np.float32)
        for h in range(HG):
            wv_ext[:, h * 65:h * 65 + 64] = Wv[:, HD3 * hg + h * 64:HD3 * hg + (h + 1) * 64]
            bv_ext[0, h * 65:h * 65 + 64] = bv[HD3 * hg + h * 64:HD3 * hg + (h + 1) * 64]
            bv_ext[0, h * 65 + 64] = 1.0
        in_maps.append({
            "xt": np.ascontiguousarray(x[b].T).astype(BF16),
            "wq": ext(Wq[:, sl] * SCALE).astype(BF16),
            "wk": ext(Wk[:, sl]).astype(BF16),
            "wv": wv_ext.astype(BF16),
            "wo": Wo[sl, :].astype(BF16),
            "bq": ext(bq[None, sl] * SCALE).astype(BF16),
            "bk": ext(bk[None, sl]).astype(BF16),
            "bv": bv_ext.astype(BF16),
        })
    return in_maps


def kernel(x, Wq, bq, Wk, bk, Wv, bv, Wo, bo):
    from concourse import bass_utils

    x = np.asarray(x, np.float32)
    Wq = np.asarray(Wq, np.float32); bq = np.asarray(bq, np.float32)
    Wk = np.asarray(Wk, np.float32); bk = np.asarray(bk, np.float32)
    Wv = np.asarray(Wv, np.float32); bv = np.asarray(bv, np.float32)
    Wo = np.asarray(Wo, np.float32); bo = np.asarray(bo, np.float32)

    nc = _get_program()
    in_maps = _make_in_maps(x, Wq, bq, Wk, bk, Wv, bv, Wo)
    _cache["in_maps"] = in_maps
    res = bass_utils.run_bass_kernel_spmd(nc, in_maps, core_ids=list(range(N_CORES)))
    _cache["last_results"] = res

    out = np.zeros((B, N, DIM), np.float32)
    for c in range(N_CORES):
        out[c // 4] += res.results[c]["out"]
    out += bo[None, None, :]
    return out


# revision 12
# speedup vs baseline: 1.3022x; 1.0860x over previous
"""Multi-head attention (B=2, N=4096, D=768, H=12) on 8 NeuronCores.

Sharding: core c -> (batch b = c//4, head-group hg = c%4 of 3 heads).
Each core computes Q/K/V projections for its 3 heads from the transposed
input xT (bf16), per-head scores^T = K @ Q^T with keys on partitions,
softmax (exp on ScalarE, denominator via a ones-column folded into the
AV matmul), AV, and the output projection restricted to its heads' rows
of Wo, producing a [4096, 768] fp32 partial. The host sums the four
head-group partials per batch and adds bo (the row-parallel all-reduce
done at unshard time).

PE row-group packing: heads 0+1 share combined [128, N] K^T/Q^T tiles so
their score matmuls run concurrently in disjoint row groups; head 2's
K^T/Q^T rows are duplicated into both halves (via host-duplicated weight
columns) so its score matmuls pair across two query blocks. Half-array
matmuls otherwise leave the PE HAM clock gate at 4/8 (1.2 GHz).

Emission is software-pipelined: K/V projections interleave with the
first attention pass (the exp stream on ScalarE is the co-bottleneck,
so PE-only projection work is overlapped with it), Q projections for
later query blocks interleave with earlier ones, and each query-block
pair's output projection is deferred into the next pair's score loop so
it never waits on the softmax-normalize chain.
"""

import numpy as np
import ml_dtypes

DIM = 768
NUM_HEADS = 12
HEAD_DIM = 64
SCALE = HEAD_DIM ** -0.5
B = 2
N = 4096
N_CORES = 8
HG = 3               # heads per core
HD3 = HG * HEAD_DIM  # 192
BF16 = ml_dtypes.bfloat16

_cache = {}


def _build_program():
    import concourse.mybir as mybir
    import concourse.tile as tile
    from concourse import bacc

    fp32 = mybir.dt.float32
    bf16 = mybir.dt.bfloat16
    AF = mybir.ActivationFunctionType

    nc = bacc.Bacc("TRN2", target_bir_lowering=False, debug=False,
                   num_devices=N_CORES)

    # wq/wk carry 256 columns: [h0|h1|h2|h2] (head 2 duplicated)
    xt_d = nc.dram_tensor("xt", [DIM, N], bf16, kind="ExternalInput")
    wq_d = nc.dram_tensor("wq", [DIM, 256], bf16, kind="ExternalInput")
    wk_d = nc.dram_tensor("wk", [DIM, 256], bf16, kind="ExternalInput")
    wv_d = nc.dram_tensor("wv", [DIM, HG * 65], bf16, kind="ExternalInput")
    wo_d = nc.dram_tensor("wo", [HD3, DIM], bf16, kind="ExternalInput")
    bq_d = nc.dram_tensor("bq", [128, 2], fp32, kind="ExternalInput")
    bk_d = nc.dram_tensor("bk", [128, 2], fp32, kind="ExternalInput")
    bv_d = nc.dram_tensor("bv", [1, HG * 65], bf16, kind="ExternalInput")
    out_d = nc.dram_tensor("out", [N, DIM], fp32, kind="ExternalOutput")

    KC = DIM // 128      # 6 contraction chunks
    NQB = N // 512       # 8 query blocks of 512
    NKB = N // 128       # 32 key blocks of 128
    V_W = HG * 65        # 195: v columns incl. per-head ones column

    with tile.TileContext(nc) as tc:
        with (
            tc.tile_pool(name="const", bufs=1) as cpool,
            tc.tile_pool(name="big", bufs=1) as bpool,
            tc.tile_pool(name="work", bufs=4) as wpool,
            tc.tile_pool(name="psum", bufs=2, space="PSUM") as ppool,
        ):
            # ---- DMAs in first-use order -----------------------------------
            wq, wk, wv, xt = [], [], [], []
            for k in range(KC):
                t = cpool.tile([128, 256], bf16, tag=f"wq{k}")
                nc.sync.dma_start(t[:], wq_d.ap()[k * 128:(k + 1) * 128, :])
                wq.append(t)
                t = cpool.tile([128, 256], bf16, tag=f"wk{k}")
                nc.sync.dma_start(t[:], wk_d.ap()[k * 128:(k + 1) * 128, :])
                wk.append(t)
                t = cpool.tile([128, V_W], bf16, tag=f"wv{k}")
                nc.sync.dma_start(t[:], wv_d.ap()[k * 128:(k + 1) * 128, :])
                wv.append(t)
            bqc = cpool.tile([128, 2], fp32, tag="bqc")
            nc.sync.dma_start(bqc[:], bq_d.ap()[:])
            bkc = cpool.tile([128, 2], fp32, tag="bkc")
            nc.sync.dma_start(bkc[:], bk_d.ap()[:])
            bv = cpool.tile([1, V_W], bf16, tag="bv")
            nc.sync.dma_start(bv[:], bv_d.ap()[:])
            ones = cpool.tile([1, 512], bf16, tag="ones")
            nc.gpsimd.memset(ones[:], 1.0)
            for k in range(KC):
                t = cpool.tile([128, N], bf16, tag=f"xt{k}")
                nc.sync.dma_start(t[:, 0:1024],
                                  xt_d.ap()[k * 128:(k + 1) * 128, 0:1024])
                xt.append(t)
            wo = []
            for h in range(HG):
                t = cpool.tile([128, DIM], bf16, tag=f"wo{h}", name=f"wo{h}")
                nc.sync.dma_start(t[0:64, :], wo_d.ap()[h * 64:(h + 1) * 64, :])
                nc.gpsimd.memset(t[64:128, :], 0.0)
                wo.append(t)
            for q in range(1, 4):
                for k in range(KC):
                    nc.sync.dma_start(
                        xt[k][:, q * 1024:(q + 1) * 1024],
                        xt_d.ap()[k * 128:(k + 1) * 128, q * 1024:(q + 1) * 1024])

            # persistent tiles
            kt01 = bpool.tile([128, N], bf16, tag="kt01")
            qt01 = bpool.tile([128, N], bf16, tag="qt01")
            kt2 = bpool.tile([128, N], bf16, tag="kt2")
            qt2 = bpool.tile([128, N], bf16, tag="qt2")
            v_sb = bpool.tile([128, NKB * V_W], bf16, tag="v")
            attn = []
            for h in range(HG):
                t = bpool.tile([128, N], bf16, tag=f"attn{h}", name=f"attn{h}")
                nc.gpsimd.memset(t[64:128, :], 0.0)
                attn.append(t)

            # ---- building blocks -------------------------------------------
            def kq_proj(nb, w, bias, dst01, dst2):
                csl = slice(nb * 512, (nb + 1) * 512)
                for m, dst in ((0, dst01), (1, dst2)):
                    ps = ppool.tile([128, 512], fp32, tag="proj")
                    for k in range(KC):
                        nc.tensor.matmul(ps[:], w[k][:, m * 128:(m + 1) * 128],
                                         xt[k][:, csl],
                                         start=(k == 0), stop=(k == KC - 1))
                    nc.vector.tensor_scalar_add(dst[:, csl], ps[:],
                                                bias[:, m:m + 1])

            def v_proj(s):
                ssl = slice(s * 128, (s + 1) * 128)
                ps = ppool.tile([128, V_W], fp32, tag="proj")
                for k in range(KC):
                    nc.tensor.matmul(ps[:], xt[k][:, ssl], wv[k][:],
                                     start=(k == 0), stop=False)
                nc.tensor.matmul(ps[:], ones[:, 0:128], bv[:],
                                 start=False, stop=True)
                nc.vector.tensor_copy(v_sb[:, s * V_W:(s + 1) * V_W], ps[:])

            def vsl(kb, h):
                return slice(kb * V_W + h * 65, kb * V_W + h * 65 + 65)

            def h01_kb(kb, qsl, av0, av1):
                ksl = slice(kb * 128, (kb + 1) * 128)
                sc = ppool.tile([128, 1024], fp32, tag="scores")
                nc.tensor.matmul(sc[:, 0:512], kt01[0:64, ksl],
                                 qt01[0:64, qsl], start=True, stop=True)
                nc.tensor.matmul(sc[:, 512:1024], kt01[64:128, ksl],
                                 qt01[64:128, qsl], start=True, stop=True)
                probs = wpool.tile([128, 1024], bf16, tag="probs")
                nc.scalar.activation(probs[:], sc[:], AF.Exp)
                nc.tensor.matmul(av0[:], v_sb[:, vsl(kb, 0)], probs[:, 0:512],
                                 start=(kb == 0), stop=(kb == NKB - 1))
                nc.tensor.matmul(av1[:], v_sb[:, vsl(kb, 1)], probs[:, 512:1024],
                                 start=(kb == 0), stop=(kb == NKB - 1))

            def h2_kb(kb, qsl0, qsl1, av2, av2b):
                ksl = slice(kb * 128, (kb + 1) * 128)
                sc = ppool.tile([128, 1024], fp32, tag="scores")
                nc.tensor.matmul(sc[:, 0:512], kt2[0:64, ksl],
                                 qt2[0:64, qsl0], start=True, stop=True)
                nc.tensor.matmul(sc[:, 512:1024], kt2[64:128, ksl],
                                 qt2[64:128, qsl1], start=True, stop=True)
                probs = wpool.tile([128, 1024], bf16, tag="probs")
                nc.scalar.activation(probs[:], sc[:], AF.Exp)
                nc.tensor.matmul(av2[:], v_sb[:, vsl(kb, 2)], probs[:, 0:512],
                                 start=(kb == 0), stop=(kb == NKB - 1))
                nc.tensor.matmul(av2b[:], v_sb[:, vsl(kb, 2)], probs[:, 512:1024],
                                 start=(kb == 0), stop=(kb == NKB - 1))

            def unload(av):
                # free the PSUM slot quickly; DVE FIFO stays short here
                av_sb = wpool.tile([65, 512], fp32, tag="av_sb")
                nc.vector.tensor_copy(av_sb[:], av[:])
                return av_sb

            def finish_norm(av_sb, h, qsl):
                r_row = wpool.tile([1, 512], fp32, tag="r_row")
                nc.vector.reciprocal(r_row[:], av_sb[64:65, :])
                r_bc = wpool.tile([64, 512], fp32, tag="r_bc")
                nc.gpsimd.partition_broadcast(r_bc[:], r_row[:])
                nc.vector.tensor_mul(attn[h][0:64, qsl], av_sb[0:64, :], r_bc[:])

            def out_proj(s):
                ssl = slice(s * 128, (s + 1) * 128)
                ob = wpool.tile([128, DIM], fp32, tag="out_sb")
                for n2 in range(2):
                    nsl = slice(n2 * 384, (n2 + 1) * 384)
                    ps = ppool.tile([128, 384], fp32, tag="proj")
                    for h in range(HG):
                        nc.tensor.matmul(ps[:], attn[h][:, ssl], wo[h][:, nsl],
                                         start=(h == 0), stop=(h == HG - 1))
                    nc.vector.tensor_copy(ob[:, nsl], ps[:])
                nc.sync.dma_start(out_d.ap()[ssl, :], ob[:])

            # ---- schedule --------------------------------------------------
            # Q^T for the first pair's query blocks
            kq_proj(0, wq, bqc, qt01, qt2)
            kq_proj(1, wq, bqc, qt01, qt2)

            # first attention pass interleaved with K^T and V production
            qsl0 = slice(0, 512)
            qsl1 = slice(512, 1024)
            av0 = ppool.tile([65, 512], fp32, tag="av")
            av1 = ppool.tile([65, 512], fp32, tag="av")
            for j in range(8):
                kq_proj(j, wk, bkc, kt01, kt2)
                for s in range(4 * j, 4 * j + 4):
                    v_proj(s)
                for kb in range(4 * j, 4 * j + 4):
                    h01_kb(kb, qsl0, av0, av1)
            sb0, sb1 = unload(av0), unload(av1)

            # remaining Q^T interleaved with the second query block's pass
            av0b = ppool.tile([65, 512], fp32, tag="av")
            av1b = ppool.tile([65, 512], fp32, tag="av")
            for kb in range(NKB):
                if kb % 6 == 0 and 2 + kb // 6 < NQB:
                    kq_proj(2 + kb // 6, wq, bqc, qt01, qt2)
                h01_kb(kb, qsl1, av0b, av1b)
            finish_norm(sb0, 0, qsl0)
            finish_norm(sb1, 1, qsl0)
            sb0b, sb1b = unload(av0b), unload(av1b)

            av2 = ppool.tile([65, 512], fp32, tag="av")
            av2b = ppool.tile([65, 512], fp32, tag="av")
            for kb in range(NKB):
                h2_kb(kb, qsl0, qsl1, av2, av2b)
            finish_norm(sb0b, 0, qsl1)
            finish_norm(sb1b, 1, qsl1)
            sb2, sb2b = unload(av2), unload(av2b)
            finish_norm(sb2, 2, qsl0)
            finish_norm(sb2b, 2, qsl1)

            # remaining pairs; out-proj of the previous pair rides along
            for qp in range(1, NQB // 2):
                qsl0 = slice((2 * qp) * 512, (2 * qp + 1) * 512)
                qsl1 = slice((2 * qp + 1) * 512, (2 * qp + 2) * 512)
                prev_s = list(range(8 * (qp - 1), 8 * qp))
                av0 = ppool.tile([65, 512], fp32, tag="av")
                av1 = ppool.tile([65, 512], fp32, tag="av")
                for kb in range(NKB):
                    if kb % 4 == 0:
                        out_proj(prev_s[kb // 4])
                    h01_kb(kb, qsl0, av0, av1)
                for s in prev_s[8:]:
                    out_proj(s)
                sb0, sb1 = unload(av0), unload(av1)
                av0b = ppool.tile([65, 512], fp32, tag="av")
                av1b = ppool.tile([65, 512], fp32, tag="av")
                for kb in range(NKB):
                    h01_kb(kb, qsl1, av0b, av1b)
                finish_norm(sb0, 0, qsl0)
                finish_norm(sb1, 1, qsl0)
                sb0b, sb1b = unload(av0b), unload(av1b)
                av2 = ppool.tile([65, 512], fp32, tag="av")
                av2b = ppool.tile([65, 512], fp32, tag="av")
                for kb in range(NKB):
                    h2_kb(kb, qsl0, qsl1, av2, av2b)
                finish_norm(sb0b, 0, qsl1)
                finish_norm(sb1b, 1, qsl1)
                sb2, sb2b = unload(av2), unload(av2b)
                finish_norm(sb2, 2, qsl0)
                finish_norm(sb2b, 2, qsl1)

            for s in range(24, 32):
                out_proj(s)

    nc.compile()
    return nc


def _get_program():
    if "nc" not in _cache:
        _cache["nc"] = _build_program()
    return _cache["nc"]


def _make_in_maps(x, Wq, bq, Wk, bk, Wv, bv, Wo):
    in_maps = []
    for c in range(N_CORES):
        b, hg = divmod(c, 4)
        sl = slice(HD3 * hg, HD3 * (hg + 1))
        # [h0|h1|h2|h2]: head 2 duplicated into both PE row-group halves
        def ext(W_sl):
            return np.concatenate([W_sl, W_sl[..., 128:192]], axis=-1)
        def bias_cols(b_ext):
            return np.stack([b_ext[0:128], b_ext[128:256]], axis=1)
        wv_ext = np.zeros((DIM, HG * 65), np.float32)
        bv_ext = np.zeros((1, HG * 65), np.float32)
        for h in range(HG):
            wv_ext[:, h * 65:h * 65 + 64] = Wv[:, HD3 * hg + h * 64:HD3 * hg + (h + 1) * 64]
            bv_ext[0, h * 65:h * 65 + 64] = bv[HD3 * hg + h * 64:HD3 * hg + (h + 1) * 64]
            bv_ext[0, h * 65 + 64] = 1.0
        in_maps.append({
            "xt": np.ascontiguousarray(x[b].T).astype(BF16),
            "wq": ext(Wq[:, sl] * SCALE).astype(BF16),
            "wk": ext(Wk[:, sl]).astype(BF16),
            "wv": wv_ext.astype(BF16),
            "wo": Wo[sl, :].astype(BF16),
            "bq": np.ascontiguousarray(bias_cols(ext(bq[sl] * SCALE)), np.float32),
            "bk": np.ascontiguousarray(bias_cols(ext(bk[sl])), np.float32),
            "bv": bv_ext.astype(BF16),
        })
    return in_maps


def kernel(x, Wq, bq, Wk, bk, Wv, bv, Wo, bo):
    from concourse import bass_utils

    x = np.asarray(x, np.float32)
    Wq = np.asarray(Wq, np.float32); bq = np.asarray(bq, np.float32)
    Wk = np.asarray(Wk, np.float32); bk = np.asarray(bk, np.float32)
    Wv = np.asarray(Wv, np.float32); bv = np.asarray(bv, np.float32)
    Wo = np.asarray(Wo, np.float32); bo = np.asarray(bo, np.float32)

    nc = _get_program()
    in_maps = _make_in_maps(x, Wq, bq, Wk, bk, Wv, bv, Wo)
    _cache["in_maps"] = in_maps
    res = bass_utils.run_bass_kernel_spmd(nc, in_maps, core_ids=list(range(N_CORES)))
    _cache["last_results"] = res

    out = np.zeros((B, N, DIM), np.float32)
    for c in range(N_CORES):
        out[c // 4] += res.results[c]["out"]
    out += bo[None, None, :]
    return out


# revision 15
# speedup vs baseline: 1.3101x; 1.0060x over previous
"""Multi-head attention (B=2, N=4096, D=768, H=12) on 8 NeuronCores.

Sharding: core c -> (batch b = c//4, head-group hg = c%4 of 3 heads).
Each core computes Q/K/V projections for its 3 heads from the transposed
input xT (bf16), per-head scores^T = K @ Q^T with keys on partitions,
softmax (exp on ScalarE, denominator via a ones-column folded into the
AV matmul), AV, and the output projection restricted to its heads' rows
of Wo, producing a [4096, 768] fp32 partial. The host sums the four
head-group partials per batch and adds bo (the row-parallel all-reduce
done at unshard time).

PE row-group packing: heads 0+1 share combined [128, N] K^T/Q^T tiles so
their score matmuls run concurrently in disjoint row groups; head 2's
K^T/Q^T rows are duplicated into both halves (via host-duplicated weight
columns) so its score matmuls pair across two query blocks. Half-array
matmuls otherwise leave the PE HAM clock gate at 4/8 (1.2 GHz).

Emission is software-pipelined: K/V projections interleave with the
first attention pass (the exp stream on ScalarE is the co-bottleneck,
so PE-only projection work is overlapped with it), Q projections for
later query blocks interleave with earlier ones, and each query-block
pair's output projection is deferred into the next pair's score loop so
it never waits on the softmax-normalize chain.
"""

import numpy as np
import ml_dtypes

DIM = 768
NUM_HEADS = 12
HEAD_DIM = 64
SCALE = HEAD_DIM ** -0.5
B = 2
N = 4096
N_CORES = 8
HG = 3               # heads per core
HD3 = HG * HEAD_DIM  # 192
BF16 = ml_dtypes.bfloat16

_cache = {}


def _build_program():
    import concourse.mybir as mybir
    import concourse.tile as tile
    from concourse import bacc

    fp32 = mybir.dt.float32
    bf16 = mybir.dt.bfloat16
    AF = mybir.ActivationFunctionType

    nc = bacc.Bacc("TRN2", target_bir_lowering=False, debug=False,
                   num_devices=N_CORES)

    # wq/wk carry 256 columns: [h0|h1|h2|h2] (head 2 duplicated)
    xt_d = nc.dram_tensor("xt", [DIM, N], bf16, kind="ExternalInput")
    wq_d = nc.dram_tensor("wq", [DIM, 256], bf16, kind="ExternalInput")
    wk_d = nc.dram_tensor("wk", [DIM, 256], bf16, kind="ExternalInput")
    wv_d = nc.dram_tensor("wv", [DIM, HG * 65], bf16, kind="ExternalInput")
    wo_d = nc.dram_tensor("wo", [HD3, DIM], bf16, kind="ExternalInput")
    bq_d = nc.dram_tensor("bq", [128, 2], fp32, kind="ExternalInput")
    bk_d = nc.dram_tensor("bk", [128, 2], fp32, kind="ExternalInput")
    bv_d = nc.dram_tensor("bv", [1, HG * 65], bf16, kind="ExternalInput")
    out_d = nc.dram_tensor("out", [N, DIM], fp32, kind="ExternalOutput")

    KC = DIM // 128      # 6 contraction chunks
    NQB = N // 512       # 8 query blocks of 512
    NKB = N // 128       # 32 key blocks of 128
    V_W = HG * 65        # 195: v columns incl. per-head ones column

    with tile.TileContext(nc) as tc:
        with (
            tc.tile_pool(name="const", bufs=1) as cpool,
            tc.tile_pool(name="big", bufs=1) as bpool,
            tc.tile_pool(name="work", bufs=4) as wpool,
            tc.tile_pool(name="psum", bufs=2, space="PSUM") as ppool,
        ):
            # ---- DMAs in first-use order -----------------------------------
            wq, wk, wv, xt = [], [], [], []
            for k in range(KC):
                t = cpool.tile([128, 256], bf16, tag=f"wq{k}")
                nc.sync.dma_start(t[:], wq_d.ap()[k * 128:(k + 1) * 128, :])
                wq.append(t)
                t = cpool.tile([128, N], bf16, tag=f"xt{k}")
                nc.sync.dma_start(t[:, 0:1024],
                                  xt_d.ap()[k * 128:(k + 1) * 128, 0:1024])
                xt.append(t)
            bqc = cpool.tile([128, 2], fp32, tag="bqc")
            nc.sync.dma_start(bqc[:], bq_d.ap()[:])
            for k in range(KC):
                t = cpool.tile([128, 256], bf16, tag=f"wk{k}")
                nc.sync.dma_start(t[:], wk_d.ap()[k * 128:(k + 1) * 128, :])
                wk.append(t)
                t = cpool.tile([128, V_W], bf16, tag=f"wv{k}")
                nc.sync.dma_start(t[:], wv_d.ap()[k * 128:(k + 1) * 128, :])
                wv.append(t)
            bkc = cpool.tile([128, 2], fp32, tag="bkc")
            nc.sync.dma_start(bkc[:], bk_d.ap()[:])
            bv = cpool.tile([1, V_W], bf16, tag="bv")
            nc.sync.dma_start(bv[:], bv_d.ap()[:])
            ones = cpool.tile([1, 512], bf16, tag="ones")
            nc.gpsimd.memset(ones[:], 1.0)
            wo = []
            for h in range(HG):
                t = cpool.tile([128, DIM], bf16, tag=f"wo{h}", name=f"wo{h}")
                nc.sync.dma_start(t[0:64, :], wo_d.ap()[h * 64:(h + 1) * 64, :])
                nc.gpsimd.memset(t[64:128, :], 0.0)
                wo.append(t)
            # bulk of xT via the SWDGE queue to keep the HWDGE queue clear
            for q in range(1, 4):
                for k in range(KC):
                    nc.gpsimd.dma_start(
                        xt[k][:, q * 1024:(q + 1) * 1024],
                        xt_d.ap()[k * 128:(k + 1) * 128, q * 1024:(q + 1) * 1024])

            # persistent tiles
            kt01 = bpool.tile([128, N], bf16, tag="kt01")
            qt01 = bpool.tile([128, N], bf16, tag="qt01")
            kt2 = bpool.tile([128, N], bf16, tag="kt2")
            qt2 = bpool.tile([128, N], bf16, tag="qt2")
            v_sb = bpool.tile([128, NKB * V_W], bf16, tag="v")
            attn = []
            for h in range(HG):
                t = bpool.tile([128, N], bf16, tag=f"attn{h}", name=f"attn{h}")
                nc.gpsimd.memset(t[64:128, :], 0.0)
                attn.append(t)

            # ---- building blocks -------------------------------------------
            def kq_proj(nb, w, bias, dst01, dst2):
                csl = slice(nb * 512, (nb + 1) * 512)
                for m, dst in ((0, dst01), (1, dst2)):
                    ps = ppool.tile([128, 512], fp32, tag="proj")
                    for k in range(KC):
                        nc.tensor.matmul(ps[:], w[k][:, m * 128:(m + 1) * 128],
                                         xt[k][:, csl],
                                         start=(k == 0), stop=(k == KC - 1))
                    nc.vector.tensor_scalar_add(dst[:, csl], ps[:],
                                                bias[:, m:m + 1])

            def v_proj(s):
                ssl = slice(s * 128, (s + 1) * 128)
                ps = ppool.tile([128, V_W], fp32, tag="proj")
                for k in range(KC):
                    nc.tensor.matmul(ps[:], xt[k][:, ssl], wv[k][:],
                                     start=(k == 0), stop=False)
                nc.tensor.matmul(ps[:], ones[:, 0:128], bv[:],
                                 start=False, stop=True)
                nc.vector.tensor_copy(v_sb[:, s * V_W:(s + 1) * V_W], ps[:])

            def vsl(kb, h):
                return slice(kb * V_W + h * 65, kb * V_W + h * 65 + 65)

            def h01_kb(kb, qsl, av0, av1):
                ksl = slice(kb * 128, (kb + 1) * 128)
                sc = ppool.tile([128, 1024], fp32, tag="scores")
                nc.tensor.matmul(sc[:, 0:512], kt01[0:64, ksl],
                                 qt01[0:64, qsl], start=True, stop=True)
                nc.tensor.matmul(sc[:, 512:1024], kt01[64:128, ksl],
                                 qt01[64:128, qsl], start=True, stop=True)
                probs = wpool.tile([128, 1024], bf16, tag="probs")
                nc.scalar.activation(probs[:], sc[:], AF.Exp)
                nc.tensor.matmul(av0[:], v_sb[:, vsl(kb, 0)], probs[:, 0:512],
                                 start=(kb == 0), stop=(kb == NKB - 1))
                nc.tensor.matmul(av1[:], v_sb[:, vsl(kb, 1)], probs[:, 512:1024],
                                 start=(kb == 0), stop=(kb == NKB - 1))

            def h2_kb(kb, qsl0, qsl1, av2, av2b):
                ksl = slice(kb * 128, (kb + 1) * 128)
                sc = ppool.tile([128, 1024], fp32, tag="scores")
                nc.tensor.matmul(sc[:, 0:512], kt2[0:64, ksl],
                                 qt2[0:64, qsl0], start=True, stop=True)
                nc.tensor.matmul(sc[:, 512:1024], kt2[64:128, ksl],
                                 qt2[64:128, qsl1], start=True, stop=True)
                probs = wpool.tile([128, 1024], bf16, tag="probs")
                nc.scalar.activation(probs[:], sc[:], AF.Exp)
                nc.tensor.matmul(av2[:], v_sb[:, vsl(kb, 2)], probs[:, 0:512],
                                 start=(kb == 0), stop=(kb == NKB - 1))
                nc.tensor.matmul(av2b[:], v_sb[:, vsl(kb, 2)], probs[:, 512:1024],
                                 start=(kb == 0), stop=(kb == NKB - 1))

            def unload(av):
                # free the PSUM slot quickly; DVE FIFO stays short here
                av_sb = wpool.tile([65, 512], fp32, tag="av_sb")
                nc.vector.tensor_copy(av_sb[:], av[:])
                return av_sb

            def finish_norm(av_sb, h, qsl):
                r_row = wpool.tile([1, 512], fp32, tag="r_row")
                nc.vector.reciprocal(r_row[:], av_sb[64:65, :])
                r_bc = wpool.tile([64, 512], fp32, tag="r_bc")
                nc.gpsimd.partition_broadcast(r_bc[:], r_row[:])
                nc.vector.tensor_mul(attn[h][0:64, qsl], av_sb[0:64, :], r_bc[:])

            def out_proj(s):
                ssl = slice(s * 128, (s + 1) * 128)
                ob = wpool.tile([128, DIM], fp32, tag="out_sb")
                for n2 in range(2):
                    nsl = slice(n2 * 384, (n2 + 1) * 384)
                    ps = ppool.tile([128, 384], fp32, tag="proj")
                    for h in range(HG):
                        nc.tensor.matmul(ps[:], attn[h][:, ssl], wo[h][:, nsl],
                                         start=(h == 0), stop=(h == HG - 1))
                    nc.vector.tensor_copy(ob[:, nsl], ps[:])
                nc.sync.dma_start(out_d.ap()[ssl, :], ob[:])

            # ---- schedule --------------------------------------------------
            # Q^T for the first pair's query blocks
            kq_proj(0, wq, bqc, qt01, qt2)
            kq_proj(1, wq, bqc, qt01, qt2)

            # first attention pass interleaved with K^T and V production
            qsl0 = slice(0, 512)
            qsl1 = slice(512, 1024)
            av0 = ppool.tile([65, 512], fp32, tag="av")
            av1 = ppool.tile([65, 512], fp32, tag="av")
            for j in range(8):
                kq_proj(j, wk, bkc, kt01, kt2)
                for s in range(4 * j, 4 * j + 4):
                    v_proj(s)
                for kb in range(4 * j, 4 * j + 4):
                    h01_kb(kb, qsl0, av0, av1)
            sb0, sb1 = unload(av0), unload(av1)

            # remaining Q^T interleaved with the second query block's pass
            av0b = ppool.tile([65, 512], fp32, tag="av")
            av1b = ppool.tile([65, 512], fp32, tag="av")
            for kb in range(NKB):
                if kb % 6 == 0 and 2 + kb // 6 < NQB:
                    kq_proj(2 + kb // 6, wq, bqc, qt01, qt2)
                h01_kb(kb, qsl1, av0b, av1b)
                if kb == 0:
                    finish_norm(sb0, 0, qsl0)
                    finish_norm(sb1, 1, qsl0)
            sb0b, sb1b = unload(av0b), unload(av1b)

            av2 = ppool.tile([65, 512], fp32, tag="av")
            av2b = ppool.tile([65, 512], fp32, tag="av")
            for kb in range(NKB):
                h2_kb(kb, qsl0, qsl1, av2, av2b)
                if kb == 0:
                    finish_norm(sb0b, 0, qsl1)
                    finish_norm(sb1b, 1, qsl1)
            sb2, sb2b = unload(av2), unload(av2b)
            finish_norm(sb2, 2, qsl0)
            finish_norm(sb2b, 2, qsl1)

            # remaining pairs; out-proj of the previous pair rides along
            for qp in range(1, NQB // 2):
                qsl0 = slice((2 * qp) * 512, (2 * qp + 1) * 512)
                qsl1 = slice((2 * qp + 1) * 512, (2 * qp + 2) * 512)
                prev_s = list(range(8 * (qp - 1), 8 * qp))
                av0 = ppool.tile([65, 512], fp32, tag="av")
                av1 = ppool.tile([65, 512], fp32, tag="av")
                for kb in range(NKB):
                    h01_kb(kb, qsl0, av0, av1)
                    if kb % 4 == 3 and kb // 4 < len(prev_s):
                        out_proj(prev_s[kb // 4])
                sb0, sb1 = unload(av0), unload(av1)
                av0b = ppool.tile([65, 512], fp32, tag="av")
                av1b = ppool.tile([65, 512], fp32, tag="av")
                for kb in range(NKB):
                    h01_kb(kb, qsl1, av0b, av1b)
                    if kb == 0:
                        finish_norm(sb0, 0, qsl0)
                        finish_norm(sb1, 1, qsl0)
                sb0b, sb1b = unload(av0b), unload(av1b)
                av2 = ppool.tile([65, 512], fp32, tag="av")
                av2b = ppool.tile([65, 512], fp32, tag="av")
                for kb in range(NKB):
                    h2_kb(kb, qsl0, qsl1, av2, av2b)
                    if kb == 0:
                        finish_norm(sb0b, 0, qsl1)
                        finish_norm(sb1b, 1, qsl1)
                sb2, sb2b = unload(av2), unload(av2b)
                finish_norm(sb2, 2, qsl0)
                finish_norm(sb2b, 2, qsl1)

            for s in range(24, 32):
                out_proj(s)

    nc.compile()
    return nc


def _get_program():
    if "nc" not in _cache:
        _cache["nc"] = _build_program()
    return _cache["nc"]


def _make_in_maps(x, Wq, bq, Wk, bk, Wv, bv, Wo):
    in_maps = []
    for c in range(N_CORES):
        b, hg = divmod(c, 4)
        sl = slice(HD3 * hg, HD3 * (hg + 1))
        # [h0|h1|h2|h2]: head 2 duplicated into both PE row-group halves
        def ext(W_sl):
            return np.concatenate([W_sl, W_sl[..., 128:192]], axis=-1)
        def bias_cols(b_ext):
            return np.stack([b_ext[0:128], b_ext[128:256]], axis=1)
        wv_ext = np.zeros((DIM, HG * 65), np.float32)
        bv_ext = np.zeros((1, HG * 65), np.float32)
        for h in range(HG):
            wv_ext[:, h * 65:h * 65 + 64] = Wv[:, HD3 * hg + h * 64:HD3 * hg + (h + 1) * 64]
            bv_ext[0, h * 65:h * 65 + 64] = bv[HD3 * hg + h * 64:HD3 * hg + (h + 1) * 64]
            bv_ext[0, h * 65 + 64] = 1.0
        in_maps.append({
            "xt": np.ascontiguousarray(x[b].T).astype(BF16),
            "wq": ext(Wq[:, sl] * SCALE).astype(BF16),
            "wk": ext(Wk[:, sl]).astype(BF16),
            "wv": wv_ext.astype(BF16),
            "wo": Wo[sl, :].astype(BF16),
            "bq": np.ascontiguousarray(bias_cols(ext(bq[sl] * SCALE)), np.float32),
            "bk": np.ascontiguousarray(bias_cols(ext(bk[sl])), np.float32),
            "bv": bv_ext.astype(BF16),
        })
    return in_maps


def kernel(x, Wq, bq, Wk, bk, Wv, bv, Wo, bo):
    from concourse import bass_utils

    x = np.asarray(x, np.float32)
    Wq = np.asarray(Wq, np.float32); bq = np.asarray(bq, np.float32)
    Wk = np.asarray(Wk, np.float32); bk = np.asarray(bk, np.float32)
    Wv = np.asarray(Wv, np.float32); bv = np.asarray(bv, np.float32)
    Wo = np.asarray(Wo, np.float32); bo = np.asarray(bo, np.float32)

    nc = _get_program()
    in_maps = _make_in_maps(x, Wq, bq, Wk, bk, Wv, bv, Wo)
    _cache["in_maps"] = in_maps
    res = bass_utils.run_bass_kernel_spmd(nc, in_maps, core_ids=list(range(N_CORES)))
    _cache["last_results"] = res

    out = np.zeros((B, N, DIM), np.float32)
    for c in range(N_CORES):
        out[c // 4] += res.results[c]["out"]
    out += bo[None, None, :]
    return out


# revision 19
# speedup vs baseline: 1.3126x; 1.0019x over previous
"""Multi-head attention (B=2, N=4096, D=768, H=12) on 8 NeuronCores.

Sharding: core c -> (batch b = c//4, head-group hg = c%4 of 3 heads).
Each core computes Q/K/V projections for its 3 heads from the transposed
input xT (bf16), per-head scores^T = K @ Q^T with keys on partitions,
softmax (exp on ScalarE, denominator via a ones-column folded into the
AV matmul), AV, and the output projection restricted to its heads' rows
of Wo, producing a [4096, 768] fp32 partial. The host sums the four
head-group partials per batch and adds bo (the row-parallel all-reduce
done at unshard time).

PE row-group packing: heads 0+1 share combined [128, N] K^T/Q^T tiles so
their score matmuls run concurrently in disjoint row groups; head 2's
K^T/Q^T rows are duplicated into both halves (via host-duplicated weight
columns) so its score matmuls pair across two query blocks. Half-array
matmuls otherwise leave the PE HAM clock gate at 4/8 (1.2 GHz).

Emission is software-pipelined: K/V projections interleave with the
first attention pass (the exp stream on ScalarE is the co-bottleneck,
so PE-only projection work is overlapped with it), Q projections for
later query blocks interleave with earlier ones, and each query-block
pair's output projection is deferred into the next pair's score loop so
it never waits on the softmax-normalize chain.
"""

import numpy as np
import ml_dtypes

DIM = 768
NUM_HEADS = 12
HEAD_DIM = 64
SCALE = HEAD_DIM ** -0.5
B = 2
N = 4096
N_CORES = 8
HG = 3               # heads per core
HD3 = HG * HEAD_DIM  # 192
BF16 = ml_dtypes.bfloat16

_cache = {}


def _build_program():
    import concourse.mybir as mybir
    import concourse.tile as tile
    from concourse import bacc

    fp32 = mybir.dt.float32
    bf16 = mybir.dt.bfloat16
    AF = mybir.ActivationFunctionType

    nc = bacc.Bacc("TRN2", target_bir_lowering=False, debug=False,
                   num_devices=N_CORES)

    # wq/wk carry 256 columns: [h0|h1|h2|h2] (head 2 duplicated)
    xt_d = nc.dram_tensor("xt", [DIM, N], bf16, kind="ExternalInput")
    wq_d = nc.dram_tensor("wq", [DIM, 256], bf16, kind="ExternalInput")
    wk_d = nc.dram_tensor("wk", [DIM, 256], bf16, kind="ExternalInput")
    wv_d = nc.dram_tensor("wv", [DIM, HG * 65], bf16, kind="ExternalInput")
    wo_d = nc.dram_tensor("wo", [HD3, DIM], bf16, kind="ExternalInput")
    bq_d = nc.dram_tensor("bq", [128, 2], fp32, kind="ExternalInput")
    bk_d = nc.dram_tensor("bk", [128, 2], fp32, kind="ExternalInput")
    bv_d = nc.dram_tensor("bv", [1, HG * 65], bf16, kind="ExternalInput")
    out_d = nc.dram_tensor("out", [N, DIM], fp32, kind="ExternalOutput")

    KC = DIM // 128      # 6 contraction chunks
    NQB = N // 512       # 8 query blocks of 512
    NKB = N // 128       # 32 key blocks of 128
    V_W = HG * 65        # 195: v columns incl. per-head ones column

    with tile.TileContext(nc) as tc:
        with (
            tc.tile_pool(name="const", bufs=1) as cpool,
            tc.tile_pool(name="big", bufs=1) as bpool,
            tc.tile_pool(name="work", bufs=4) as wpool,
            tc.tile_pool(name="psum", bufs=2, space="PSUM") as ppool,
        ):
            # ---- DMAs in first-use order -----------------------------------
            wq, wk, wv, xt = [], [], [], []
            for k in range(KC):
                t = cpool.tile([128, 256], bf16, tag=f"wq{k}")
                nc.sync.dma_start(t[:], wq_d.ap()[k * 128:(k + 1) * 128, :])
                wq.append(t)
                t = cpool.tile([128, N], bf16, tag=f"xt{k}")
                nc.sync.dma_start(t[:, 0:1024],
                                  xt_d.ap()[k * 128:(k + 1) * 128, 0:1024])
                xt.append(t)
            bqc = cpool.tile([128, 2], fp32, tag="bqc")
            nc.sync.dma_start(bqc[:], bq_d.ap()[:])
            for k in range(KC):
                t = cpool.tile([128, 256], bf16, tag=f"wk{k}")
                nc.sync.dma_start(t[:], wk_d.ap()[k * 128:(k + 1) * 128, :])
                wk.append(t)
                t = cpool.tile([128, V_W], bf16, tag=f"wv{k}")
                nc.sync.dma_start(t[:], wv_d.ap()[k * 128:(k + 1) * 128, :])
                wv.append(t)
            bkc = cpool.tile([128, 2], fp32, tag="bkc")
            nc.sync.dma_start(bkc[:], bk_d.ap()[:])
            bv = cpool.tile([1, V_W], bf16, tag="bv")
            nc.sync.dma_start(bv[:], bv_d.ap()[:])
            ones = cpool.tile([1, 512], bf16, tag="ones")
            nc.gpsimd.memset(ones[:], 1.0)
            wo = []
            for h in range(HG):
                t = cpool.tile([128, DIM], bf16, tag=f"wo{h}", name=f"wo{h}")
                nc.sync.dma_start(t[0:64, :], wo_d.ap()[h * 64:(h + 1) * 64, :])
                nc.gpsimd.memset(t[64:128, :], 0.0)
                wo.append(t)
            # bulk of xT via the SWDGE queue to keep the HWDGE queue clear
            for q in range(1, 4):
                for k in range(KC):
                    nc.gpsimd.dma_start(
                        xt[k][:, q * 1024:(q + 1) * 1024],
                        xt_d.ap()[k * 128:(k + 1) * 128, q * 1024:(q + 1) * 1024])

            # persistent tiles
            kt01 = bpool.tile([128, N], bf16, tag="kt01")
            qt01 = bpool.tile([128, N], bf16, tag="qt01")
            kt2 = bpool.tile([128, N], bf16, tag="kt2")
            qt2 = bpool.tile([128, N], bf16, tag="qt2")
            v_sb = bpool.tile([128, NKB * V_W], bf16, tag="v")
            attn = []
            for h in range(HG):
                t = bpool.tile([128, N], bf16, tag=f"attn{h}", name=f"attn{h}")
                nc.gpsimd.memset(t[64:128, :], 0.0)
                attn.append(t)

            # ---- building blocks -------------------------------------------
            def kq_proj(nb, w, bias, dst01, dst2):
                csl = slice(nb * 512, (nb + 1) * 512)
                for m, dst in ((0, dst01), (1, dst2)):
                    ps = ppool.tile([128, 512], fp32, tag="proj")
                    for k in range(KC):
                        nc.tensor.matmul(ps[:], w[k][:, m * 128:(m + 1) * 128],
                                         xt[k][:, csl],
                                         start=(k == 0), stop=(k == KC - 1))
                    nc.vector.tensor_scalar_add(dst[:, csl], ps[:],
                                                bias[:, m:m + 1])

            def v_proj(s):
                ssl = slice(s * 128, (s + 1) * 128)
                ps = ppool.tile([128, V_W], fp32, tag="proj")
                for k in range(KC):
                    nc.tensor.matmul(ps[:], xt[k][:, ssl], wv[k][:],
                                     start=(k == 0), stop=False)
                nc.tensor.matmul(ps[:], ones[:, 0:128], bv[:],
                                 start=False, stop=True)
                nc.vector.tensor_copy(v_sb[:, s * V_W:(s + 1) * V_W], ps[:])

            def vsl(kb, h):
                return slice(kb * V_W + h * 65, kb * V_W + h * 65 + 65)

            def h01_kb(kb, qsl, av0, av1):
                ksl = slice(kb * 128, (kb + 1) * 128)
                sc = ppool.tile([128, 1024], fp32, tag="scores")
                nc.tensor.matmul(sc[:, 0:512], kt01[0:64, ksl],
                                 qt01[0:64, qsl], start=True, stop=True)
                nc.tensor.matmul(sc[:, 512:1024], kt01[64:128, ksl],
                                 qt01[64:128, qsl], start=True, stop=True)
                probs = wpool.tile([128, 1024], bf16, tag="probs")
                nc.scalar.activation(probs[:], sc[:], AF.Exp)
                nc.tensor.matmul(av0[:], v_sb[:, vsl(kb, 0)], probs[:, 0:512],
                                 start=(kb == 0), stop=(kb == NKB - 1))
                nc.tensor.matmul(av1[:], v_sb[:, vsl(kb, 1)], probs[:, 512:1024],
                                 start=(kb == 0), stop=(kb == NKB - 1))

            def h2_kb(kb, qsl0, qsl1, av2, av2b):
                ksl = slice(kb * 128, (kb + 1) * 128)
                sc = ppool.tile([128, 1024], fp32, tag="scores")
                nc.tensor.matmul(sc[:, 0:512], kt2[0:64, ksl],
                                 qt2[0:64, qsl0], start=True, stop=True)
                nc.tensor.matmul(sc[:, 512:1024], kt2[64:128, ksl],
                                 qt2[64:128, qsl1], start=True, stop=True)
                probs = wpool.tile([128, 1024], bf16, tag="probs")
                nc.scalar.activation(probs[:], sc[:], AF.Exp)
                nc.tensor.matmul(av2[:], v_sb[:, vsl(kb, 2)], probs[:, 0:512],
                                 start=(kb == 0), stop=(kb == NKB - 1))
                nc.tensor.matmul(av2b[:], v_sb[:, vsl(kb, 2)], probs[:, 512:1024],
                                 start=(kb == 0), stop=(kb == NKB - 1))

            def unload(av):
                # free the PSUM slot quickly; DVE FIFO stays short here
                av_sb = wpool.tile([65, 512], fp32, tag="av_sb")
                nc.vector.tensor_copy(av_sb[:], av[:])
                return av_sb

            def finish_norm(av_sb, h, qsl):
                r_row = wpool.tile([1, 512], fp32, tag="r_row")
                nc.vector.reciprocal(r_row[:], av_sb[64:65, :])
                r_bc = wpool.tile([64, 512], fp32, tag="r_bc")
                nc.gpsimd.partition_broadcast(r_bc[:], r_row[:])
                nc.vector.tensor_mul(attn[h][0:64, qsl], av_sb[0:64, :], r_bc[:])

            def out_proj(s):
                ssl = slice(s * 128, (s + 1) * 128)
                ob = wpool.tile([128, DIM], fp32, tag="out_sb")
                for n2 in range(2):
                    nsl = slice(n2 * 384, (n2 + 1) * 384)
                    ps = ppool.tile([128, 384], fp32, tag="proj")
                    for h in range(HG):
                        nc.tensor.matmul(ps[:], attn[h][:, ssl], wo[h][:, nsl],
                                         start=(h == 0), stop=(h == HG - 1))
                    nc.vector.tensor_copy(ob[:, nsl], ps[:])
                nc.sync.dma_start(out_d.ap()[ssl, :], ob[:])

            # ---- schedule --------------------------------------------------
            # Q^T for the first pair's query blocks
            kq_proj(0, wq, bqc, qt01, qt2)
            kq_proj(1, wq, bqc, qt01, qt2)

            # first attention pass interleaved with K^T and V production
            qsl0 = slice(0, 512)
            qsl1 = slice(512, 1024)
            av0 = ppool.tile([65, 512], fp32, tag="av")
            av1 = ppool.tile([65, 512], fp32, tag="av")
            for j in range(8):
                kq_proj(j, wk, bkc, kt01, kt2)
                for s in range(4 * j, 4 * j + 4):
                    v_proj(s)
                for kb in range(4 * j, 4 * j + 4):
                    h01_kb(kb, qsl0, av0, av1)
            sb0, sb1 = unload(av0), unload(av1)

            # remaining Q^T interleaved with the second query block's pass
            av0b = ppool.tile([65, 512], fp32, tag="av")
            av1b = ppool.tile([65, 512], fp32, tag="av")
            for kb in range(NKB):
                if kb % 6 == 0 and 2 + kb // 6 < NQB:
                    kq_proj(2 + kb // 6, wq, bqc, qt01, qt2)
                h01_kb(kb, qsl1, av0b, av1b)
                if kb == 0:
                    finish_norm(sb0, 0, qsl0)
                    finish_norm(sb1, 1, qsl0)
            sb0b, sb1b = unload(av0b), unload(av1b)

            av2 = ppool.tile([65, 512], fp32, tag="av")
            av2b = ppool.tile([65, 512], fp32, tag="av")
            for kb in range(NKB):
                h2_kb(kb, qsl0, qsl1, av2, av2b)
                if kb == 0:
                    finish_norm(sb0b, 0, qsl1)
                    finish_norm(sb1b, 1, qsl1)
            sb2, sb2b = unload(av2), unload(av2b)
            finish_norm(sb2, 2, qsl0)
            finish_norm(sb2b, 2, qsl1)

            # remaining pairs; out-proj of the previous pair rides along
            for qp in range(1, NQB // 2):
                qsl0 = slice((2 * qp) * 512, (2 * qp + 1) * 512)
                qsl1 = slice((2 * qp + 1) * 512, (2 * qp + 2) * 512)
                prev_s = list(range(8 * (qp - 1), 8 * qp))
                av0 = ppool.tile([65, 512], fp32, tag="av")
                av1 = ppool.tile([65, 512], fp32, tag="av")
                for kb in range(NKB):
                    h01_kb(kb, qsl0, av0, av1)
                    # deferred out-proj: start late enough that the previous
                    # pair's normalize chain (recips on DVE) has drained
                    if kb >= 11 and (kb - 11) % 3 == 0 and (kb - 11) // 3 < 7:
                        out_proj(prev_s[(kb - 11) // 3])
                out_proj(prev_s[7])
                sb0, sb1 = unload(av0), unload(av1)
                av0b = ppool.tile([65, 512], fp32, tag="av")
                av1b = ppool.tile([65, 512], fp32, tag="av")
                for kb in range(NKB):
                    h01_kb(kb, qsl1, av0b, av1b)
                    if kb == 0:
                        finish_norm(sb0, 0, qsl0)
                        finish_norm(sb1, 1, qsl0)
                sb0b, sb1b = unload(av0b), unload(av1b)
                av2 = ppool.tile([65, 512], fp32, tag="av")
                av2b = ppool.tile([65, 512], fp32, tag="av")
                for kb in range(NKB):
                    h2_kb(kb, qsl0, qsl1, av2, av2b)
                    if kb == 0:
                        finish_norm(sb0b, 0, qsl1)
                        finish_norm(sb1b, 1, qsl1)
                sb2, sb2b = unload(av2), unload(av2b)
                if qp < NQB // 2 - 1:
                    finish_norm(sb2, 2, qsl0)
                    finish_norm(sb2b, 2, qsl1)

            # last pair: pipeline the final out-projections against the
            # trailing head-2 normalizes
            finish_norm(sb2, 2, slice(3072, 3584))
            for s in range(24, 28):
                out_proj(s)
            finish_norm(sb2b, 2, slice(3584, 4096))
            for s in range(28, 32):
                out_proj(s)

    nc.compile()
    return nc


def _get_program():
    if "nc" not in _cache:
        _cache["nc"] = _build_program()
    return _cache["nc"]


def _make_in_maps(x, Wq, bq, Wk, bk, Wv, bv, Wo):
    in_maps = []
    for c in range(N_CORES):
        b, hg = divmod(c, 4)
        sl = slice(HD3 * hg, HD3 * (hg + 1))
        # [h0|h1|h2|h2]: head 2 duplicated into both PE row-group halves
        def ext(W_sl):
            return np.concatenate([W_sl, W_sl[..., 128:192]], axis=-1)
        def bias_cols(b_ext):
            return np.stack([b_ext[0:128], b_ext[128:256]], axis=1)
        wv_ext = np.zeros((DIM, HG * 65), np.float32)
        bv_ext = np.zeros((1, HG * 65), np.float32)
        for h in range(HG):
            wv_ext[:, h * 65:h * 65 + 64] = Wv[:, HD3 * hg + h * 64:HD3 * hg + (h + 1) * 64]
            bv_ext[0, h * 65:h * 65 + 64] = bv[HD3 * hg + h * 64:HD3 * hg + (h + 1) * 64]
            bv_ext[0, h * 65 + 64] = 1.0
        in_maps.append({
            "xt": np.ascontiguousarray(x[b].T).astype(BF16),
            "wq": ext(Wq[:, sl] * SCALE).astype(BF16),
            "wk": ext(Wk[:, sl]).astype(BF16),
            "wv": wv_ext.astype(BF16),
            "wo": Wo[sl, :].astype(BF16),
            "bq": np.ascontiguousarray(bias_cols(ext(bq[sl] * SCALE)), np.float32),
            "bk": np.ascontiguousarray(bias_cols(ext(bk[sl])), np.float32),
            "bv": bv_ext.astype(BF16),
        })
    return in_maps


def kernel(x, Wq, bq, Wk, bk, Wv, bv, Wo, bo):
    from concourse import bass_utils

    x = np.asarray(x, np.float32)
    Wq = np.asarray(Wq, np.float32); bq = np.asarray(bq, np.float32)
    Wk = np.asarray(Wk, np.float32); bk = np.asarray(bk, np.float32)
    Wv = np.asarray(Wv, np.float32); bv = np.asarray(bv, np.float32)
    Wo = np.asarray(Wo, np.float32); bo = np.asarray(bo, np.float32)

    nc = _get_program()
    in_maps = _make_in_maps(x, Wq, bq, Wk, bk, Wv, bv, Wo)
    _cache["in_maps"] = in_maps
    res = bass_utils.run_bass_kernel_spmd(nc, in_maps, core_ids=list(range(N_CORES)))
    _cache["last_results"] = res

    out = np.zeros((B, N, DIM), np.float32)
    for c in range(N_CORES):
        out[c // 4] += res.results[c]["out"]
    out += bo[None, None, :]
    return out


# revision 20
# speedup vs baseline: 1.3403x; 1.0211x over previous
"""Multi-head attention (B=2, N=4096, D=768, H=12) on 8 NeuronCores.

Sharding: core c -> (batch b = c//4, head-group hg = c%4 of 3 heads).
Each core computes Q/K/V projections for its 3 heads from the transposed
input xT (bf16), per-head scores^T = K @ Q^T with keys on partitions,
softmax (exp on ScalarE, denominator via a ones-column folded into the
AV matmul), AV, and the output projection restricted to its heads' rows
of Wo, producing a [4096, 768] fp32 partial. The host sums the four
head-group partials per batch and adds bo (the row-parallel all-reduce
done at unshard time).

PE row-group packing: heads 0+1 share combined [128, N] K^T/Q^T tiles so
their score matmuls run concurrently in disjoint row groups; head 2's
K^T/Q^T rows are duplicated into both halves (via host-duplicated weight
columns) so its score matmuls pair across two query blocks. Half-array
matmuls otherwise leave the PE HAM clock gate at 4/8 (1.2 GHz).

Emission is software-pipelined: K/V projections interleave with the
first attention pass (the exp stream on ScalarE is the co-bottleneck,
so PE-only projection work is overlapped with it), Q projections for
later query blocks interleave with earlier ones, and each query-block
pair's output projection is deferred into the next pair's score loop so
it never waits on the softmax-normalize chain.
"""

import numpy as np
import ml_dtypes

DIM = 768
NUM_HEADS = 12
HEAD_DIM = 64
SCALE = HEAD_DIM ** -0.5
B = 2
N = 4096
N_CORES = 8
HG = 3               # heads per core
HD3 = HG * HEAD_DIM  # 192
BF16 = ml_dtypes.bfloat16

_cache = {}


def _build_program():
    import concourse.mybir as mybir
    import concourse.tile as tile
    from concourse import bacc
    from concourse.tile_rust import add_dep_helper

    fp32 = mybir.dt.float32
    bf16 = mybir.dt.bfloat16
    AF = mybir.ActivationFunctionType

    nc = bacc.Bacc("TRN2", target_bir_lowering=False, debug=False,
                   num_devices=N_CORES)

    # wq/wk carry 256 columns: [h0|h1|h2|h2] (head 2 duplicated)
    xt_d = nc.dram_tensor("xt", [DIM, N], bf16, kind="ExternalInput")
    wq_d = nc.dram_tensor("wq", [DIM, 256], bf16, kind="ExternalInput")
    wk_d = nc.dram_tensor("wk", [DIM, 256], bf16, kind="ExternalInput")
    wv_d = nc.dram_tensor("wv", [DIM, HG * 65], bf16, kind="ExternalInput")
    wo_d = nc.dram_tensor("wo", [HD3, DIM], bf16, kind="ExternalInput")
    bq_d = nc.dram_tensor("bq", [128, 2], fp32, kind="ExternalInput")
    bk_d = nc.dram_tensor("bk", [128, 2], fp32, kind="ExternalInput")
    bv_d = nc.dram_tensor("bv", [1, HG * 65], bf16, kind="ExternalInput")
    out_d = nc.dram_tensor("out", [N, DIM], fp32, kind="ExternalOutput")

    KC = DIM // 128      # 6 contraction chunks
    NQB = N // 512       # 8 query blocks of 512
    NKB = N // 128       # 32 key blocks of 128
    V_W = HG * 65        # 195: v columns incl. per-head ones column

    with tile.TileContext(nc) as tc:
        with (
            tc.tile_pool(name="const", bufs=1) as cpool,
            tc.tile_pool(name="big", bufs=1) as bpool,
            tc.tile_pool(name="work", bufs=4) as wpool,
            tc.tile_pool(name="psum", bufs=2, space="PSUM") as ppool,
        ):
            # ---- DMAs in first-use order -----------------------------------
            wq, wk, wv, xt = [], [], [], []
            for k in range(KC):
                t = cpool.tile([128, 256], bf16, tag=f"wq{k}")
                nc.sync.dma_start(t[:], wq_d.ap()[k * 128:(k + 1) * 128, :])
                wq.append(t)
                t = cpool.tile([128, N], bf16, tag=f"xt{k}")
                nc.sync.dma_start(t[:, 0:1024],
                                  xt_d.ap()[k * 128:(k + 1) * 128, 0:1024])
                xt.append(t)
            bqc = cpool.tile([128, 2], fp32, tag="bqc")
            nc.sync.dma_start(bqc[:], bq_d.ap()[:])
            for k in range(KC):
                t = cpool.tile([128, 256], bf16, tag=f"wk{k}")
                nc.sync.dma_start(t[:], wk_d.ap()[k * 128:(k + 1) * 128, :])
                wk.append(t)
                t = cpool.tile([128, V_W], bf16, tag=f"wv{k}")
                nc.sync.dma_start(t[:], wv_d.ap()[k * 128:(k + 1) * 128, :])
                wv.append(t)
            bkc = cpool.tile([128, 2], fp32, tag="bkc")
            nc.sync.dma_start(bkc[:], bk_d.ap()[:])
            bv = cpool.tile([1, V_W], bf16, tag="bv")
            nc.sync.dma_start(bv[:], bv_d.ap()[:])
            ones = cpool.tile([1, 512], bf16, tag="ones")
            nc.gpsimd.memset(ones[:], 1.0)
            wo = []
            for h in range(HG):
                t = cpool.tile([128, DIM], bf16, tag=f"wo{h}", name=f"wo{h}")
                nc.sync.dma_start(t[0:64, :], wo_d.ap()[h * 64:(h + 1) * 64, :])
                nc.gpsimd.memset(t[64:128, :], 0.0)
                wo.append(t)
            # bulk of xT via the SWDGE queue to keep the HWDGE queue clear
            for q in range(1, 4):
                for k in range(KC):
                    nc.gpsimd.dma_start(
                        xt[k][:, q * 1024:(q + 1) * 1024],
                        xt_d.ap()[k * 128:(k + 1) * 128, q * 1024:(q + 1) * 1024])

            # persistent tiles
            kt01 = bpool.tile([128, N], bf16, tag="kt01")
            qt01 = bpool.tile([128, N], bf16, tag="qt01")
            kt2 = bpool.tile([128, N], bf16, tag="kt2")
            qt2 = bpool.tile([128, N], bf16, tag="qt2")
            v_sb = bpool.tile([128, NKB * V_W], bf16, tag="v")
            attn = []
            for h in range(HG):
                t = bpool.tile([128, N], bf16, tag=f"attn{h}", name=f"attn{h}")
                nc.gpsimd.memset(t[64:128, :], 0.0)
                attn.append(t)

            # ---- building blocks -------------------------------------------
            def kq_proj(nb, w, bias, dst01, dst2):
                csl = slice(nb * 512, (nb + 1) * 512)
                for m, dst in ((0, dst01), (1, dst2)):
                    ps = ppool.tile([128, 512], fp32, tag="proj")
                    for k in range(KC):
                        nc.tensor.matmul(ps[:], w[k][:, m * 128:(m + 1) * 128],
                                         xt[k][:, csl],
                                         start=(k == 0), stop=(k == KC - 1))
                    nc.vector.tensor_scalar_add(dst[:, csl], ps[:],
                                                bias[:, m:m + 1])

            def v_proj(s):
                ssl = slice(s * 128, (s + 1) * 128)
                ps = ppool.tile([128, V_W], fp32, tag="proj")
                for k in range(KC):
                    nc.tensor.matmul(ps[:], xt[k][:, ssl], wv[k][:],
                                     start=(k == 0), stop=False)
                nc.tensor.matmul(ps[:], ones[:, 0:128], bv[:],
                                 start=False, stop=True)
                nc.vector.tensor_copy(v_sb[:, s * V_W:(s + 1) * V_W], ps[:])

            def vsl(kb, h):
                return slice(kb * V_W + h * 65, kb * V_W + h * 65 + 65)

            def h01_kb(kb, qsl, av0, av1):
                ksl = slice(kb * 128, (kb + 1) * 128)
                sc = ppool.tile([128, 1024], fp32, tag="scores")
                nc.tensor.matmul(sc[:, 0:512], kt01[0:64, ksl],
                                 qt01[0:64, qsl], start=True, stop=True)
                nc.tensor.matmul(sc[:, 512:1024], kt01[64:128, ksl],
                                 qt01[64:128, qsl], start=True, stop=True)
                probs = wpool.tile([128, 1024], bf16, tag="probs")
                e = nc.scalar.activation(probs[:], sc[:], AF.Exp)
                nc.tensor.matmul(av0[:], v_sb[:, vsl(kb, 0)], probs[:, 0:512],
                                 start=(kb == 0), stop=(kb == NKB - 1))
                nc.tensor.matmul(av1[:], v_sb[:, vsl(kb, 1)], probs[:, 512:1024],
                                 start=(kb == 0), stop=(kb == NKB - 1))
                return e

            def h2_kb(kb, qsl0, qsl1, av2, av2b):
                ksl = slice(kb * 128, (kb + 1) * 128)
                sc = ppool.tile([128, 1024], fp32, tag="scores")
                nc.tensor.matmul(sc[:, 0:512], kt2[0:64, ksl],
                                 qt2[0:64, qsl0], start=True, stop=True)
                nc.tensor.matmul(sc[:, 512:1024], kt2[64:128, ksl],
                                 qt2[64:128, qsl1], start=True, stop=True)
                probs = wpool.tile([128, 1024], bf16, tag="probs")
                nc.scalar.activation(probs[:], sc[:], AF.Exp)
                nc.tensor.matmul(av2[:], v_sb[:, vsl(kb, 2)], probs[:, 0:512],
                                 start=(kb == 0), stop=(kb == NKB - 1))
                nc.tensor.matmul(av2b[:], v_sb[:, vsl(kb, 2)], probs[:, 512:1024],
                                 start=(kb == 0), stop=(kb == NKB - 1))

            def unload(av):
                # free the PSUM slot quickly; DVE FIFO stays short here
                av_sb = wpool.tile([65, 512], fp32, tag="av_sb")
                nc.vector.tensor_copy(av_sb[:], av[:])
                return av_sb

            def finish_norm(av_sb, h, qsl):
                r_row = wpool.tile([1, 512], fp32, tag="r_row")
                nc.vector.reciprocal(r_row[:], av_sb[64:65, :])
                r_bc = wpool.tile([64, 512], fp32, tag="r_bc")
                nc.gpsimd.partition_broadcast(r_bc[:], r_row[:])
                return nc.vector.tensor_mul(attn[h][0:64, qsl],
                                            av_sb[0:64, :], r_bc[:])

            def out_proj(s, after=None):
                ssl = slice(s * 128, (s + 1) * 128)
                ob = wpool.tile([128, DIM], fp32, tag="out_sb")
                for n2 in range(2):
                    nsl = slice(n2 * 384, (n2 + 1) * 384)
                    ps = ppool.tile([128, 384], fp32, tag="proj")
                    for h in range(HG):
                        mm = nc.tensor.matmul(ps[:], attn[h][:, ssl],
                                              wo[h][:, nsl],
                                              start=(h == 0), stop=(h == HG - 1))
                        if after is not None:
                            add_dep_helper(mm.ins, after.ins, sync=True,
                                           reason="pin deferred out_proj")
                            after = None
                    nc.vector.tensor_copy(ob[:, nsl], ps[:])
                nc.sync.dma_start(out_d.ap()[ssl, :], ob[:])

            # ---- schedule --------------------------------------------------
            # Q^T for the first pair's query blocks
            kq_proj(0, wq, bqc, qt01, qt2)
            kq_proj(1, wq, bqc, qt01, qt2)

            # first attention pass interleaved with K^T and V production
            qsl0 = slice(0, 512)
            qsl1 = slice(512, 1024)
            av0 = ppool.tile([65, 512], fp32, tag="av")
            av1 = ppool.tile([65, 512], fp32, tag="av")
            for j in range(8):
                kq_proj(j, wk, bkc, kt01, kt2)
                for s in range(4 * j, 4 * j + 4):
                    v_proj(s)
                for kb in range(4 * j, 4 * j + 4):
                    h01_kb(kb, qsl0, av0, av1)
            sb0, sb1 = unload(av0), unload(av1)

            # remaining Q^T interleaved with the second query block's pass
            av0b = ppool.tile([65, 512], fp32, tag="av")
            av1b = ppool.tile([65, 512], fp32, tag="av")
            for kb in range(NKB):
                if kb % 6 == 0 and 2 + kb // 6 < NQB:
                    kq_proj(2 + kb // 6, wq, bqc, qt01, qt2)
                h01_kb(kb, qsl1, av0b, av1b)
                if kb == 0:
                    finish_norm(sb0, 0, qsl0)
                    finish_norm(sb1, 1, qsl0)
            sb0b, sb1b = unload(av0b), unload(av1b)

            av2 = ppool.tile([65, 512], fp32, tag="av")
            av2b = ppool.tile([65, 512], fp32, tag="av")
            for kb in range(NKB):
                h2_kb(kb, qsl0, qsl1, av2, av2b)
                if kb == 0:
                    finish_norm(sb0b, 0, qsl1)
                    finish_norm(sb1b, 1, qsl1)
            sb2, sb2b = unload(av2), unload(av2b)
            finish_norm(sb2, 2, qsl0)
            finish_norm(sb2b, 2, qsl1)

            # remaining pairs; out-proj of the previous pair rides along
            for qp in range(1, NQB // 2):
                qsl0 = slice((2 * qp) * 512, (2 * qp + 1) * 512)
                qsl1 = slice((2 * qp + 1) * 512, (2 * qp + 2) * 512)
                prev_s = list(range(8 * (qp - 1), 8 * qp))
                av0 = ppool.tile([65, 512], fp32, tag="av")
                av1 = ppool.tile([65, 512], fp32, tag="av")
                for kb in range(NKB):
                    e = h01_kb(kb, qsl0, av0, av1)
                    # deferred out-proj: start late enough that the previous
                    # pair's normalize chain (recips on DVE) has drained
                    if kb >= 11 and (kb - 11) % 3 == 0 and (kb - 11) // 3 < 7:
                        out_proj(prev_s[(kb - 11) // 3], after=e)
                out_proj(prev_s[7], after=e)
                sb0, sb1 = unload(av0), unload(av1)
                av0b = ppool.tile([65, 512], fp32, tag="av")
                av1b = ppool.tile([65, 512], fp32, tag="av")
                for kb in range(NKB):
                    h01_kb(kb, qsl1, av0b, av1b)
                    if kb == 0:
                        finish_norm(sb0, 0, qsl0)
                        finish_norm(sb1, 1, qsl0)
                sb0b, sb1b = unload(av0b), unload(av1b)
                av2 = ppool.tile([65, 512], fp32, tag="av")
                av2b = ppool.tile([65, 512], fp32, tag="av")
                for kb in range(NKB):
                    h2_kb(kb, qsl0, qsl1, av2, av2b)
                    if kb == 0:
                        finish_norm(sb0b, 0, qsl1)
                        finish_norm(sb1b, 1, qsl1)
                sb2, sb2b = unload(av2), unload(av2b)
                if qp < NQB // 2 - 1:
                    finish_norm(sb2, 2, qsl0)
                    finish_norm(sb2b, 2, qsl1)

            # last pair: pipeline the final out-projections against the
            # trailing head-2 normalizes
            m2 = finish_norm(sb2, 2, slice(3072, 3584))
            for s in range(24, 28):
                out_proj(s, after=m2 if s == 24 else None)
            m2b = finish_norm(sb2b, 2, slice(3584, 4096))
            for s in range(28, 32):
                out_proj(s, after=m2b if s == 28 else None)

    nc.compile()
    return nc


def _get_program():
    if "nc" not in _cache:
        _cache["nc"] = _build_program()
    return _cache["nc"]


def _make_in_maps(x, Wq, bq, Wk, bk, Wv, bv, Wo):
    in_maps = []
    for c in range(N_CORES):
        b, hg = divmod(c, 4)
        sl = slice(HD3 * hg, HD3 * (hg + 1))
        # [h0|h1|h2|h2]: head 2 duplicated into both PE row-group halves
        def ext(W_sl):
            return np.concatenate([W_sl, W_sl[..., 128:192]], axis=-1)
        def bias_cols(b_ext):
            return np.stack([b_ext[0:128], b_ext[128:256]], axis=1)
        wv_ext = np.zeros((DIM, HG * 65), np.float32)
        bv_ext = np.zeros((1, HG * 65), np.float32)
        for h in range(HG):
            wv_ext[:, h * 65:h * 65 + 64] = Wv[:, HD3 * hg + h * 64:HD3 * hg + (h + 1) * 64]
            bv_ext[0, h * 65:h * 65 + 64] = bv[HD3 * hg + h * 64:HD3 * hg + (h + 1) * 64]
            bv_ext[0, h * 65 + 64] = 1.0
        in_maps.append({
            "xt": np.ascontiguousarray(x[b].T).astype(BF16),
            "wq": ext(Wq[:, sl] * SCALE).astype(BF16),
            "wk": ext(Wk[:, sl]).astype(BF16),
            "wv": wv_ext.astype(BF16),
            "wo": Wo[sl, :].astype(BF16),
            "bq": np.ascontiguousarray(bias_cols(ext(bq[sl] * SCALE)), np.float32),
            "bk": np.ascontiguousarray(bias_cols(ext(bk[sl])), np.float32),
            "bv": bv_ext.astype(BF16),
        })
    return in_maps


def kernel(x, Wq, bq, Wk, bk, Wv, bv, Wo, bo):
    from concourse import bass_utils

    x = np.asarray(x, np.float32)
    Wq = np.asarray(Wq, np.float32); bq = np.asarray(bq, np.float32)
    Wk = np.asarray(Wk, np.float32); bk = np.asarray(bk, np.float32)
    Wv = np.asarray(Wv, np.float32); bv = np.asarray(bv, np.float32)
    Wo = np.asarray(Wo, np.float32); bo = np.asarray(bo, np.float32)

    nc = _get_program()
    in_maps = _make_in_maps(x, Wq, bq, Wk, bk, Wv, bv, Wo)
    _cache["in_maps"] = in_maps
    res = bass_utils.run_bass_kernel_spmd(nc, in_maps, core_ids=list(range(N_CORES)))
    _cache["last_results"] = res

    out = np.zeros((B, N, DIM), np.float32)
    for c in range(N_CORES):
        out[c // 4] += res.results[c]["out"]
    out += bo[None, None, :]
    return out


# revision 21
# speedup vs baseline: 1.3437x; 1.0025x over previous
"""Multi-head attention (B=2, N=4096, D=768, H=12) on 8 NeuronCores.

Sharding: core c -> (batch b = c//4, head-group hg = c%4 of 3 heads).
Each core computes Q/K/V projections for its 3 heads from the transposed
input xT (bf16), per-head scores^T = K @ Q^T with keys on partitions,
softmax (exp on ScalarE, denominator via a ones-column folded into the
AV matmul), AV, and the output projection restricted to its heads' rows
of Wo, producing a [4096, 768] fp32 partial. The host sums the four
head-group partials per batch and adds bo (the row-parallel all-reduce
done at unshard time).

PE row-group packing: heads 0+1 share combined [128, N] K^T/Q^T tiles so
their score matmuls run concurrently in disjoint row groups; head 2's
K^T/Q^T rows are duplicated into both halves (via host-duplicated weight
columns) so its score matmuls pair across two query blocks. Half-array
matmuls otherwise leave the PE HAM clock gate at 4/8 (1.2 GHz).

Emission is software-pipelined: K/V projections interleave with the
first attention pass (the exp stream on ScalarE is the co-bottleneck,
so PE-only projection work is overlapped with it), Q projections for
later query blocks interleave with earlier ones, and each query-block
pair's output projection is deferred into the next pair's score loop so
it never waits on the softmax-normalize chain.
"""

import numpy as np
import ml_dtypes

DIM = 768
NUM_HEADS = 12
HEAD_DIM = 64
SCALE = HEAD_DIM ** -0.5
B = 2
N = 4096
N_CORES = 8
HG = 3               # heads per core
HD3 = HG * HEAD_DIM  # 192
BF16 = ml_dtypes.bfloat16

_cache = {}


def _build_program():
    import concourse.mybir as mybir
    import concourse.tile as tile
    from concourse import bacc
    from concourse.tile_rust import add_dep_helper

    fp32 = mybir.dt.float32
    bf16 = mybir.dt.bfloat16
    AF = mybir.ActivationFunctionType

    nc = bacc.Bacc("TRN2", target_bir_lowering=False, debug=False,
                   num_devices=N_CORES)

    # wq/wk carry 256 columns: [h0|h1|h2|h2] (head 2 duplicated)
    xt_d = nc.dram_tensor("xt", [DIM, N], bf16, kind="ExternalInput")
    wq_d = nc.dram_tensor("wq", [DIM, 256], bf16, kind="ExternalInput")
    wk_d = nc.dram_tensor("wk", [DIM, 256], bf16, kind="ExternalInput")
    wv_d = nc.dram_tensor("wv", [DIM, HG * 65], bf16, kind="ExternalInput")
    wo_d = nc.dram_tensor("wo", [HD3, DIM], bf16, kind="ExternalInput")
    bq_d = nc.dram_tensor("bq", [128, 2], fp32, kind="ExternalInput")
    bk_d = nc.dram_tensor("bk", [128, 2], fp32, kind="ExternalInput")
    bv_d = nc.dram_tensor("bv", [1, HG * 65], bf16, kind="ExternalInput")
    out_d = nc.dram_tensor("out", [N, DIM], fp32, kind="ExternalOutput")

    KC = DIM // 128      # 6 contraction chunks
    NQB = N // 512       # 8 query blocks of 512
    NKB = N // 128       # 32 key blocks of 128
    V_W = HG * 65        # 195: v columns incl. per-head ones column

    with tile.TileContext(nc) as tc:
        with (
            tc.tile_pool(name="const", bufs=1) as cpool,
            tc.tile_pool(name="big", bufs=1) as bpool,
            tc.tile_pool(name="work", bufs=6) as wpool,
            tc.tile_pool(name="psum", bufs=2, space="PSUM") as ppool,
        ):
            # ---- DMAs in first-use order -----------------------------------
            wq, wk, wv, xt = [], [], [], []
            for k in range(KC):
                t = cpool.tile([128, 256], bf16, tag=f"wq{k}")
                nc.sync.dma_start(t[:], wq_d.ap()[k * 128:(k + 1) * 128, :])
                wq.append(t)
                t = cpool.tile([128, N], bf16, tag=f"xt{k}")
                nc.sync.dma_start(t[:, 0:1024],
                                  xt_d.ap()[k * 128:(k + 1) * 128, 0:1024])
                xt.append(t)
            bqc = cpool.tile([128, 2], fp32, tag="bqc")
            nc.sync.dma_start(bqc[:], bq_d.ap()[:])
            for k in range(KC):
                t = cpool.tile([128, 256], bf16, tag=f"wk{k}")
                nc.sync.dma_start(t[:], wk_d.ap()[k * 128:(k + 1) * 128, :])
                wk.append(t)
                t = cpool.tile([128, V_W], bf16, tag=f"wv{k}")
                nc.sync.dma_start(t[:], wv_d.ap()[k * 128:(k + 1) * 128, :])
                wv.append(t)
            bkc = cpool.tile([128, 2], fp32, tag="bkc")
            nc.sync.dma_start(bkc[:], bk_d.ap()[:])
            bv = cpool.tile([1, V_W], bf16, tag="bv")
            nc.sync.dma_start(bv[:], bv_d.ap()[:])
            ones = cpool.tile([1, 512], bf16, tag="ones")
            nc.gpsimd.memset(ones[:], 1.0)
            wo = []
            for h in range(HG):
                t = cpool.tile([128, DIM], bf16, tag=f"wo{h}", name=f"wo{h}")
                nc.sync.dma_start(t[0:64, :], wo_d.ap()[h * 64:(h + 1) * 64, :])
                nc.gpsimd.memset(t[64:128, :], 0.0)
                wo.append(t)
            # bulk of xT via the SWDGE queue to keep the HWDGE queue clear
            for q in range(1, 4):
                for k in range(KC):
                    nc.gpsimd.dma_start(
                        xt[k][:, q * 1024:(q + 1) * 1024],
                        xt_d.ap()[k * 128:(k + 1) * 128, q * 1024:(q + 1) * 1024])

            # persistent tiles
            kt01 = bpool.tile([128, N], bf16, tag="kt01")
            qt01 = bpool.tile([128, N], bf16, tag="qt01")
            kt2 = bpool.tile([128, N], bf16, tag="kt2")
            qt2 = bpool.tile([128, N], bf16, tag="qt2")
            v_sb = bpool.tile([128, NKB * V_W], bf16, tag="v")
            attn = []
            for h in range(HG):
                t = bpool.tile([128, N], bf16, tag=f"attn{h}", name=f"attn{h}")
                nc.gpsimd.memset(t[64:128, :], 0.0)
                attn.append(t)

            # ---- building blocks -------------------------------------------
            def kq_proj(nb, w, bias, dst01, dst2):
                csl = slice(nb * 512, (nb + 1) * 512)
                for m, dst in ((0, dst01), (1, dst2)):
                    ps = ppool.tile([128, 512], fp32, tag="proj")
                    for k in range(KC):
                        nc.tensor.matmul(ps[:], w[k][:, m * 128:(m + 1) * 128],
                                         xt[k][:, csl],
                                         start=(k == 0), stop=(k == KC - 1))
                    nc.vector.tensor_scalar_add(dst[:, csl], ps[:],
                                                bias[:, m:m + 1])

            def v_proj(s):
                ssl = slice(s * 128, (s + 1) * 128)
                ps = ppool.tile([128, V_W], fp32, tag="proj")
                for k in range(KC):
                    nc.tensor.matmul(ps[:], xt[k][:, ssl], wv[k][:],
                                     start=(k == 0), stop=False)
                nc.tensor.matmul(ps[:], ones[:, 0:128], bv[:],
                                 start=False, stop=True)
                nc.vector.tensor_copy(v_sb[:, s * V_W:(s + 1) * V_W], ps[:])

            def vsl(kb, h):
                return slice(kb * V_W + h * 65, kb * V_W + h * 65 + 65)

            def h01_kb(kb, qsl, av0, av1):
                ksl = slice(kb * 128, (kb + 1) * 128)
                sc = ppool.tile([128, 1024], fp32, tag="scores")
                nc.tensor.matmul(sc[:, 0:512], kt01[0:64, ksl],
                                 qt01[0:64, qsl], start=True, stop=True)
                nc.tensor.matmul(sc[:, 512:1024], kt01[64:128, ksl],
                                 qt01[64:128, qsl], start=True, stop=True)
                probs = wpool.tile([128, 1024], bf16, tag="probs")
                e = nc.scalar.activation(probs[:], sc[:], AF.Exp)
                nc.tensor.matmul(av0[:], v_sb[:, vsl(kb, 0)], probs[:, 0:512],
                                 start=(kb == 0), stop=(kb == NKB - 1))
                nc.tensor.matmul(av1[:], v_sb[:, vsl(kb, 1)], probs[:, 512:1024],
                                 start=(kb == 0), stop=(kb == NKB - 1))
                return e

            def h2_kb(kb, qsl0, qsl1, av2, av2b):
                ksl = slice(kb * 128, (kb + 1) * 128)
                sc = ppool.tile([128, 1024], fp32, tag="scores")
                nc.tensor.matmul(sc[:, 0:512], kt2[0:64, ksl],
                                 qt2[0:64, qsl0], start=True, stop=True)
                nc.tensor.matmul(sc[:, 512:1024], kt2[64:128, ksl],
                                 qt2[64:128, qsl1], start=True, stop=True)
                probs = wpool.tile([128, 1024], bf16, tag="probs")
                nc.scalar.activation(probs[:], sc[:], AF.Exp)
                nc.tensor.matmul(av2[:], v_sb[:, vsl(kb, 2)], probs[:, 0:512],
                                 start=(kb == 0), stop=(kb == NKB - 1))
                nc.tensor.matmul(av2b[:], v_sb[:, vsl(kb, 2)], probs[:, 512:1024],
                                 start=(kb == 0), stop=(kb == NKB - 1))

            def unload(av):
                # free the PSUM slot quickly; DVE FIFO stays short here
                av_sb = wpool.tile([65, 512], fp32, tag="av_sb")
                nc.vector.tensor_copy(av_sb[:], av[:])
                return av_sb

            def finish_norm(av_sb, h, qsl):
                r_row = wpool.tile([1, 512], fp32, tag="r_row")
                nc.vector.reciprocal(r_row[:], av_sb[64:65, :])
                r_bc = wpool.tile([64, 512], fp32, tag="r_bc")
                nc.gpsimd.partition_broadcast(r_bc[:], r_row[:])
                return nc.vector.tensor_mul(attn[h][0:64, qsl],
                                            av_sb[0:64, :], r_bc[:])

            def out_proj(s, after=None):
                ssl = slice(s * 128, (s + 1) * 128)
                ob = wpool.tile([128, DIM], fp32, tag="out_sb")
                for n2 in range(2):
                    nsl = slice(n2 * 384, (n2 + 1) * 384)
                    ps = ppool.tile([128, 384], fp32, tag="proj")
                    for h in range(HG):
                        mm = nc.tensor.matmul(ps[:], attn[h][:, ssl],
                                              wo[h][:, nsl],
                                              start=(h == 0), stop=(h == HG - 1))
                        if after is not None:
                            add_dep_helper(mm.ins, after.ins, sync=True,
                                           reason="pin deferred out_proj")
                            after = None
                    nc.vector.tensor_copy(ob[:, nsl], ps[:])
                nc.sync.dma_start(out_d.ap()[ssl, :], ob[:])

            # ---- schedule --------------------------------------------------
            # Q^T for the first pair's query blocks
            kq_proj(0, wq, bqc, qt01, qt2)
            kq_proj(1, wq, bqc, qt01, qt2)

            # first attention pass interleaved with K^T and V production
            qsl0 = slice(0, 512)
            qsl1 = slice(512, 1024)
            av0 = ppool.tile([65, 512], fp32, tag="av")
            av1 = ppool.tile([65, 512], fp32, tag="av")
            for j in range(8):
                kq_proj(j, wk, bkc, kt01, kt2)
                for s in range(4 * j, 4 * j + 4):
                    v_proj(s)
                for kb in range(4 * j, 4 * j + 4):
                    h01_kb(kb, qsl0, av0, av1)
            sb0, sb1 = unload(av0), unload(av1)

            # remaining Q^T interleaved with the second query block's pass
            av0b = ppool.tile([65, 512], fp32, tag="av")
            av1b = ppool.tile([65, 512], fp32, tag="av")
            for kb in range(NKB):
                if kb % 6 == 0 and 2 + kb // 6 < NQB:
                    kq_proj(2 + kb // 6, wq, bqc, qt01, qt2)
                h01_kb(kb, qsl1, av0b, av1b)
                if kb == 0:
                    finish_norm(sb0, 0, qsl0)
                    finish_norm(sb1, 1, qsl0)
            sb0b, sb1b = unload(av0b), unload(av1b)

            av2 = ppool.tile([65, 512], fp32, tag="av")
            av2b = ppool.tile([65, 512], fp32, tag="av")
            for kb in range(NKB):
                h2_kb(kb, qsl0, qsl1, av2, av2b)
                if kb == 0:
                    finish_norm(sb0b, 0, qsl1)
                    finish_norm(sb1b, 1, qsl1)
            sb2, sb2b = unload(av2), unload(av2b)
            finish_norm(sb2, 2, qsl0)
            finish_norm(sb2b, 2, qsl1)

            # remaining pairs; out-proj of the previous pair rides along
            for qp in range(1, NQB // 2):
                qsl0 = slice((2 * qp) * 512, (2 * qp + 1) * 512)
                qsl1 = slice((2 * qp + 1) * 512, (2 * qp + 2) * 512)
                prev_s = list(range(8 * (qp - 1), 8 * qp))
                av0 = ppool.tile([65, 512], fp32, tag="av")
                av1 = ppool.tile([65, 512], fp32, tag="av")
                for kb in range(NKB):
                    e = h01_kb(kb, qsl0, av0, av1)
                    # deferred out-proj: start late enough that the previous
                    # pair's normalize chain (recips on DVE) has drained
                    if kb >= 11 and (kb - 11) % 3 == 0 and (kb - 11) // 3 < 7:
                        out_proj(prev_s[(kb - 11) // 3], after=e)
                out_proj(prev_s[7], after=e)
                sb0, sb1 = unload(av0), unload(av1)
                av0b = ppool.tile([65, 512], fp32, tag="av")
                av1b = ppool.tile([65, 512], fp32, tag="av")
                for kb in range(NKB):
                    h01_kb(kb, qsl1, av0b, av1b)
                    if kb == 0:
                        finish_norm(sb0, 0, qsl0)
                        finish_norm(sb1, 1, qsl0)
                sb0b, sb1b = unload(av0b), unload(av1b)
                av2 = ppool.tile([65, 512], fp32, tag="av")
                av2b = ppool.tile([65, 512], fp32, tag="av")
                for kb in range(NKB):
                    h2_kb(kb, qsl0, qsl1, av2, av2b)
                    if kb == 0:
                        finish_norm(sb0b, 0, qsl1)
                        finish_norm(sb1b, 1, qsl1)
                sb2, sb2b = unload(av2), unload(av2b)
                if qp < NQB // 2 - 1:
                    finish_norm(sb2, 2, qsl0)
                    finish_norm(sb2b, 2, qsl1)

            # last pair: pipeline the final out-projections against the
            # trailing head-2 normalizes
            m2 = finish_norm(sb2, 2, slice(3072, 3584))
            for s in range(24, 28):
                out_proj(s, after=m2 if s == 24 else None)
            m2b = finish_norm(sb2b, 2, slice(3584, 4096))
            for s in range(28, 32):
                out_proj(s, after=m2b if s == 28 else None)

    nc.compile()
    return nc


def _get_program():
    if "nc" not in _cache:
        _cache["nc"] = _build_program()
    return _cache["nc"]


def _make_in_maps(x, Wq, bq, Wk, bk, Wv, bv, Wo):
    in_maps = []
    for c in range(N_CORES):
        b, hg = divmod(c, 4)
        sl = slice(HD3 * hg, HD3 * (hg + 1))
        # [h0|h1|h2|h2]: head 2 duplicated into both PE row-group halves
        def ext(W_sl):
            return np.concatenate([W_sl, W_sl[..., 128:192]], axis=-1)
        def bias_cols(b_ext):
            return np.stack([b_ext[0:128], b_ext[128:256]], axis=1)
        wv_ext = np.zeros((DIM, HG * 65), np.float32)
        bv_ext = np.zeros((1, HG * 65), np.float32)
        for h in range(HG):
            wv_ext[:, h * 65:h * 65 + 64] = Wv[:, HD3 * hg + h * 64:HD3 * hg + (h + 1) * 64]
            bv_ext[0, h * 65:h * 65 + 64] = bv[HD3 * hg + h * 64:HD3 * hg + (h + 1) * 64]
            bv_ext[0, h * 65 + 64] = 1.0
        in_maps.append({
            "xt": np.ascontiguousarray(x[b].T).astype(BF16),
            "wq": ext(Wq[:, sl] * SCALE).astype(BF16),
            "wk": ext(Wk[:, sl]).astype(BF16),
            "wv": wv_ext.astype(BF16),
            "wo": Wo[sl, :].astype(BF16),
            "bq": np.ascontiguousarray(bias_cols(ext(bq[sl] * SCALE)), np.float32),
            "bk": np.ascontiguousarray(bias_cols(ext(bk[sl])), np.float32),
            "bv": bv_ext.astype(BF16),
        })
    return in_maps


def kernel(x, Wq, bq, Wk, bk, Wv, bv, Wo, bo):
    from concourse import bass_utils

    x = np.asarray(x, np.float32)
    Wq = np.asarray(Wq, np.float32); bq = np.asarray(bq, np.float32)
    Wk = np.asarray(Wk, np.float32); bk = np.asarray(bk, np.float32)
    Wv = np.asarray(Wv, np.float32); bv = np.asarray(bv, np.float32)
    Wo = np.asarray(Wo, np.float32); bo = np.asarray(bo, np.float32)

    nc = _get_program()
    in_maps = _make_in_maps(x, Wq, bq, Wk, bk, Wv, bv, Wo)
    _cache["in_maps"] = in_maps
    res = bass_utils.run_bass_kernel_spmd(nc, in_maps, core_ids=list(range(N_CORES)))
    _cache["last_results"] = res

    out = np.zeros((B, N, DIM), np.float32)
    for c in range(N_CORES):
        out[c // 4] += res.results[c]["out"]
    out += bo[None, None, :]
    return out


# revision 22
# speedup vs baseline: 1.3442x; 1.0004x over previous
"""Multi-head attention (B=2, N=4096, D=768, H=12) on 8 NeuronCores.

Sharding: core c -> (batch b = c//4, head-group hg = c%4 of 3 heads).
Each core computes Q/K/V projections for its 3 heads from the transposed
input xT (bf16), per-head scores^T = K @ Q^T with keys on partitions,
softmax (exp on ScalarE, denominator via a ones-column folded into the
AV matmul), AV, and the output projection restricted to its heads' rows
of Wo, producing a [4096, 768] fp32 partial. The host sums the four
head-group partials per batch and adds bo (the row-parallel all-reduce
done at unshard time).

PE row-group packing: heads 0+1 share combined [128, N] K^T/Q^T tiles so
their score matmuls run concurrently in disjoint row groups; head 2's
K^T/Q^T rows are duplicated into both halves (via host-duplicated weight
columns) so its score matmuls pair across two query blocks. Half-array
matmuls otherwise leave the PE HAM clock gate at 4/8 (1.2 GHz).

Emission is software-pipelined: K/V projections interleave with the
first attention pass (the exp stream on ScalarE is the co-bottleneck,
so PE-only projection work is overlapped with it), Q projections for
later query blocks interleave with earlier ones, and each query-block
pair's output projection is deferred into the next pair's score loop so
it never waits on the softmax-normalize chain.
"""

import numpy as np
import ml_dtypes

DIM = 768
NUM_HEADS = 12
HEAD_DIM = 64
SCALE = HEAD_DIM ** -0.5
B = 2
N = 4096
N_CORES = 8
HG = 3               # heads per core
HD3 = HG * HEAD_DIM  # 192
BF16 = ml_dtypes.bfloat16

_cache = {}


def _build_program():
    import concourse.mybir as mybir
    import concourse.tile as tile
    from concourse import bacc
    from concourse.tile_rust import add_dep_helper

    fp32 = mybir.dt.float32
    bf16 = mybir.dt.bfloat16
    AF = mybir.ActivationFunctionType

    nc = bacc.Bacc("TRN2", target_bir_lowering=False, debug=False,
                   num_devices=N_CORES)

    # wq/wk carry 256 columns: [h0|h1|h2|h2] (head 2 duplicated)
    xt_d = nc.dram_tensor("xt", [DIM, N], bf16, kind="ExternalInput")
    wq_d = nc.dram_tensor("wq", [DIM, 256], bf16, kind="ExternalInput")
    wk_d = nc.dram_tensor("wk", [DIM, 256], bf16, kind="ExternalInput")
    wv_d = nc.dram_tensor("wv", [DIM, HG * 65], bf16, kind="ExternalInput")
    wo_d = nc.dram_tensor("wo", [HD3, DIM], bf16, kind="ExternalInput")
    bq_d = nc.dram_tensor("bq", [128, 2], fp32, kind="ExternalInput")
    bk_d = nc.dram_tensor("bk", [128, 2], fp32, kind="ExternalInput")
    bv_d = nc.dram_tensor("bv", [1, HG * 65], bf16, kind="ExternalInput")
    out_d = nc.dram_tensor("out", [N, DIM], fp32, kind="ExternalOutput")

    KC = DIM // 128      # 6 contraction chunks
    NQB = N // 512       # 8 query blocks of 512
    NKB = N // 128       # 32 key blocks of 128
    V_W = HG * 65        # 195: v columns incl. per-head ones column

    with tile.TileContext(nc) as tc:
        with (
            tc.tile_pool(name="const", bufs=1) as cpool,
            tc.tile_pool(name="big", bufs=1) as bpool,
            tc.tile_pool(name="work", bufs=6) as wpool,
            tc.tile_pool(name="psum", bufs=2, space="PSUM") as ppool,
        ):
            # ---- DMAs in first-use order -----------------------------------
            wq, wk, wv, xt = [], [], [], []
            for k in range(KC):
                t = cpool.tile([128, 256], bf16, tag=f"wq{k}")
                nc.sync.dma_start(t[:], wq_d.ap()[k * 128:(k + 1) * 128, :])
                wq.append(t)
                t = cpool.tile([128, N], bf16, tag=f"xt{k}")
                nc.sync.dma_start(t[:, 0:512],
                                  xt_d.ap()[k * 128:(k + 1) * 128, 0:512])
                xt.append(t)
            bqc = cpool.tile([128, 2], fp32, tag="bqc")
            nc.sync.dma_start(bqc[:], bq_d.ap()[:])
            for k in range(KC):
                nc.sync.dma_start(xt[k][:, 512:1024],
                                  xt_d.ap()[k * 128:(k + 1) * 128, 512:1024])
            for k in range(KC):
                t = cpool.tile([128, 256], bf16, tag=f"wk{k}")
                nc.sync.dma_start(t[:], wk_d.ap()[k * 128:(k + 1) * 128, :])
                wk.append(t)
                t = cpool.tile([128, V_W], bf16, tag=f"wv{k}")
                nc.sync.dma_start(t[:], wv_d.ap()[k * 128:(k + 1) * 128, :])
                wv.append(t)
            bkc = cpool.tile([128, 2], fp32, tag="bkc")
            nc.sync.dma_start(bkc[:], bk_d.ap()[:])
            bv = cpool.tile([1, V_W], bf16, tag="bv")
            nc.sync.dma_start(bv[:], bv_d.ap()[:])
            ones = cpool.tile([1, 512], bf16, tag="ones")
            nc.gpsimd.memset(ones[:], 1.0)
            wo = []
            for h in range(HG):
                t = cpool.tile([128, DIM], bf16, tag=f"wo{h}", name=f"wo{h}")
                nc.sync.dma_start(t[0:64, :], wo_d.ap()[h * 64:(h + 1) * 64, :])
                nc.gpsimd.memset(t[64:128, :], 0.0)
                wo.append(t)
            # bulk of xT via the SWDGE queue to keep the HWDGE queue clear
            for q in range(1, 4):
                for k in range(KC):
                    nc.gpsimd.dma_start(
                        xt[k][:, q * 1024:(q + 1) * 1024],
                        xt_d.ap()[k * 128:(k + 1) * 128, q * 1024:(q + 1) * 1024])

            # persistent tiles
            kt01 = bpool.tile([128, N], bf16, tag="kt01")
            qt01 = bpool.tile([128, N], bf16, tag="qt01")
            kt2 = bpool.tile([128, N], bf16, tag="kt2")
            qt2 = bpool.tile([128, N], bf16, tag="qt2")
            v_sb = bpool.tile([128, NKB * V_W], bf16, tag="v")
            attn = []
            for h in range(HG):
                t = bpool.tile([128, N], bf16, tag=f"attn{h}", name=f"attn{h}")
                nc.gpsimd.memset(t[64:128, :], 0.0)
                attn.append(t)

            # ---- building blocks -------------------------------------------
            def kq_proj(nb, w, bias, dst01, dst2):
                csl = slice(nb * 512, (nb + 1) * 512)
                for m, dst in ((0, dst01), (1, dst2)):
                    ps = ppool.tile([128, 512], fp32, tag="proj")
                    for k in range(KC):
                        nc.tensor.matmul(ps[:], w[k][:, m * 128:(m + 1) * 128],
                                         xt[k][:, csl],
                                         start=(k == 0), stop=(k == KC - 1))
                    nc.vector.tensor_scalar_add(dst[:, csl], ps[:],
                                                bias[:, m:m + 1])

            def v_proj(s):
                ssl = slice(s * 128, (s + 1) * 128)
                ps = ppool.tile([128, V_W], fp32, tag="proj")
                for k in range(KC):
                    nc.tensor.matmul(ps[:], xt[k][:, ssl], wv[k][:],
                                     start=(k == 0), stop=False)
                nc.tensor.matmul(ps[:], ones[:, 0:128], bv[:],
                                 start=False, stop=True)
                nc.vector.tensor_copy(v_sb[:, s * V_W:(s + 1) * V_W], ps[:])

            def vsl(kb, h):
                return slice(kb * V_W + h * 65, kb * V_W + h * 65 + 65)

            def h01_kb(kb, qsl, av0, av1):
                ksl = slice(kb * 128, (kb + 1) * 128)
                sc = ppool.tile([128, 1024], fp32, tag="scores")
                nc.tensor.matmul(sc[:, 0:512], kt01[0:64, ksl],
                                 qt01[0:64, qsl], start=True, stop=True)
                nc.tensor.matmul(sc[:, 512:1024], kt01[64:128, ksl],
                                 qt01[64:128, qsl], start=True, stop=True)
                probs = wpool.tile([128, 1024], bf16, tag="probs")
                e = nc.scalar.activation(probs[:], sc[:], AF.Exp)
                nc.tensor.matmul(av0[:], v_sb[:, vsl(kb, 0)], probs[:, 0:512],
                                 start=(kb == 0), stop=(kb == NKB - 1))
                nc.tensor.matmul(av1[:], v_sb[:, vsl(kb, 1)], probs[:, 512:1024],
                                 start=(kb == 0), stop=(kb == NKB - 1))
                return e

            def h2_kb(kb, qsl0, qsl1, av2, av2b):
                ksl = slice(kb * 128, (kb + 1) * 128)
                sc = ppool.tile([128, 1024], fp32, tag="scores")
                nc.tensor.matmul(sc[:, 0:512], kt2[0:64, ksl],
                                 qt2[0:64, qsl0], start=True, stop=True)
                nc.tensor.matmul(sc[:, 512:1024], kt2[64:128, ksl],
                                 qt2[64:128, qsl1], start=True, stop=True)
                probs = wpool.tile([128, 1024], bf16, tag="probs")
                nc.scalar.activation(probs[:], sc[:], AF.Exp)
                nc.tensor.matmul(av2[:], v_sb[:, vsl(kb, 2)], probs[:, 0:512],
                                 start=(kb == 0), stop=(kb == NKB - 1))
                nc.tensor.matmul(av2b[:], v_sb[:, vsl(kb, 2)], probs[:, 512:1024],
                                 start=(kb == 0), stop=(kb == NKB - 1))

            def unload(av):
                # free the PSUM slot quickly; DVE FIFO stays short here
                av_sb = wpool.tile([65, 512], fp32, tag="av_sb")
                nc.vector.tensor_copy(av_sb[:], av[:])
                return av_sb

            def finish_norm(av_sb, h, qsl):
                r_row = wpool.tile([1, 512], fp32, tag="r_row")
                nc.vector.reciprocal(r_row[:], av_sb[64:65, :])
                r_bc = wpool.tile([64, 512], fp32, tag="r_bc")
                nc.gpsimd.partition_broadcast(r_bc[:], r_row[:])
                return nc.vector.tensor_mul(attn[h][0:64, qsl],
                                            av_sb[0:64, :], r_bc[:])

            def out_proj(s, after=None):
                ssl = slice(s * 128, (s + 1) * 128)
                ob = wpool.tile([128, DIM], fp32, tag="out_sb")
                for n2 in range(2):
                    nsl = slice(n2 * 384, (n2 + 1) * 384)
                    ps = ppool.tile([128, 384], fp32, tag="proj")
                    for h in range(HG):
                        mm = nc.tensor.matmul(ps[:], attn[h][:, ssl],
                                              wo[h][:, nsl],
                                              start=(h == 0), stop=(h == HG - 1))
                        if after is not None:
                            add_dep_helper(mm.ins, after.ins, sync=True,
                                           reason="pin deferred out_proj")
                            after = None
                    nc.vector.tensor_copy(ob[:, nsl], ps[:])
                nc.sync.dma_start(out_d.ap()[ssl, :], ob[:])

            # ---- schedule --------------------------------------------------
            # Q^T for the first pair's query blocks
            kq_proj(0, wq, bqc, qt01, qt2)
            kq_proj(1, wq, bqc, qt01, qt2)

            # first attention pass interleaved with K^T and V production
            qsl0 = slice(0, 512)
            qsl1 = slice(512, 1024)
            av0 = ppool.tile([65, 512], fp32, tag="av")
            av1 = ppool.tile([65, 512], fp32, tag="av")
            for j in range(8):
                kq_proj(j, wk, bkc, kt01, kt2)
                for s in range(4 * j, 4 * j + 4):
                    v_proj(s)
                for kb in range(4 * j, 4 * j + 4):
                    h01_kb(kb, qsl0, av0, av1)
            sb0, sb1 = unload(av0), unload(av1)

            # remaining Q^T interleaved with the second query block's pass
            av0b = ppool.tile([65, 512], fp32, tag="av")
            av1b = ppool.tile([65, 512], fp32, tag="av")
            for kb in range(NKB):
                if kb % 6 == 0 and 2 + kb // 6 < NQB:
                    kq_proj(2 + kb // 6, wq, bqc, qt01, qt2)
                h01_kb(kb, qsl1, av0b, av1b)
                if kb == 0:
                    finish_norm(sb0, 0, qsl0)
                    finish_norm(sb1, 1, qsl0)
            sb0b, sb1b = unload(av0b), unload(av1b)

            av2 = ppool.tile([65, 512], fp32, tag="av")
            av2b = ppool.tile([65, 512], fp32, tag="av")
            for kb in range(NKB):
                h2_kb(kb, qsl0, qsl1, av2, av2b)
                if kb == 0:
                    finish_norm(sb0b, 0, qsl1)
                    finish_norm(sb1b, 1, qsl1)
            sb2, sb2b = unload(av2), unload(av2b)
            finish_norm(sb2, 2, qsl0)
            finish_norm(sb2b, 2, qsl1)

            # remaining pairs; out-proj of the previous pair rides along
            for qp in range(1, NQB // 2 - 1):
                qsl0 = slice((2 * qp) * 512, (2 * qp + 1) * 512)
                qsl1 = slice((2 * qp + 1) * 512, (2 * qp + 2) * 512)
                prev_s = list(range(8 * (qp - 1), 8 * qp))
                av0 = ppool.tile([65, 512], fp32, tag="av")
                av1 = ppool.tile([65, 512], fp32, tag="av")
                for kb in range(NKB):
                    e = h01_kb(kb, qsl0, av0, av1)
                    # deferred out-proj: start late enough that the previous
                    # pair's normalize chain (recips on DVE) has drained
                    if kb >= 11 and (kb - 11) % 3 == 0 and (kb - 11) // 3 < 7:
                        out_proj(prev_s[(kb - 11) // 3], after=e)
                out_proj(prev_s[7], after=e)
                sb0, sb1 = unload(av0), unload(av1)
                av0b = ppool.tile([65, 512], fp32, tag="av")
                av1b = ppool.tile([65, 512], fp32, tag="av")
                for kb in range(NKB):
                    h01_kb(kb, qsl1, av0b, av1b)
                    if kb == 0:
                        finish_norm(sb0, 0, qsl0)
                        finish_norm(sb1, 1, qsl0)
                sb0b, sb1b = unload(av0b), unload(av1b)
                av2 = ppool.tile([65, 512], fp32, tag="av")
                av2b = ppool.tile([65, 512], fp32, tag="av")
                for kb in range(NKB):
                    h2_kb(kb, qsl0, qsl1, av2, av2b)
                    if kb == 0:
                        finish_norm(sb0b, 0, qsl1)
                        finish_norm(sb1b, 1, qsl1)
                sb2, sb2b = unload(av2), unload(av2b)
                finish_norm(sb2, 2, qsl0)
                finish_norm(sb2b, 2, qsl1)

            # last pair: head-2 pass runs second so the tail only waits on
            # the second query block's normalize chain
            qsl0 = slice(3072, 3584)
            qsl1 = slice(3584, 4096)
            prev_s = list(range(16, 24))
            av0 = ppool.tile([65, 512], fp32, tag="av")
            av1 = ppool.tile([65, 512], fp32, tag="av")
            for kb in range(NKB):
                e = h01_kb(kb, qsl0, av0, av1)
                if kb >= 11 and (kb - 11) % 3 == 0 and (kb - 11) // 3 < 7:
                    out_proj(prev_s[(kb - 11) // 3], after=e)
            out_proj(prev_s[7], after=e)
            sb0, sb1 = unload(av0), unload(av1)
            av2 = ppool.tile([65, 512], fp32, tag="av")
            av2b = ppool.tile([65, 512], fp32, tag="av")
            for kb in range(NKB):
                e = h2_kb(kb, qsl0, qsl1, av2, av2b)
                if kb == 0:
                    finish_norm(sb0, 0, qsl0)
                    finish_norm(sb1, 1, qsl0)
            sb2, sb2b = unload(av2), unload(av2b)
            av0b = ppool.tile([65, 512], fp32, tag="av")
            av1b = ppool.tile([65, 512], fp32, tag="av")
            ndone = 0
            for kb in range(NKB):
                e = h01_kb(kb, qsl1, av0b, av1b)
                if kb == 0:
                    finish_norm(sb2, 2, qsl0)
                    finish_norm(sb2b, 2, qsl1)
                if kb >= 11 and (kb - 11) % 5 == 0 and ndone < 4:
                    out_proj(24 + ndone, after=e)
                    ndone += 1
            sb0b, sb1b = unload(av0b), unload(av1b)
            finish_norm(sb0b, 0, qsl1)
            m1b = finish_norm(sb1b, 1, qsl1)
            for s in range(28, 32):
                out_proj(s, after=m1b if s == 28 else None)

    nc.compile()
    return nc


def _get_program():
    if "nc" not in _cache:
        _cache["nc"] = _build_program()
    return _cache["nc"]


def _make_in_maps(x, Wq, bq, Wk, bk, Wv, bv, Wo):
    in_maps = []
    for c in range(N_CORES):
        b, hg = divmod(c, 4)
        sl = slice(HD3 * hg, HD3 * (hg + 1))
        # [h0|h1|h2|h2]: head 2 duplicated into both PE row-group halves
        def ext(W_sl):
            return np.concatenate([W_sl, W_sl[..., 128:192]], axis=-1)
        def bias_cols(b_ext):
            return np.stack([b_ext[0:128], b_ext[128:256]], axis=1)
        wv_ext = np.zeros((DIM, HG * 65), np.float32)
        bv_ext = np.zeros((1, HG * 65), np.float32)
        for h in range(HG):
            wv_ext[:, h * 65:h * 65 + 64] = Wv[:, HD3 * hg + h * 64:HD3 * hg + (h + 1) * 64]
            bv_ext[0, h * 65:h * 65 + 64] = bv[HD3 * hg + h * 64:HD3 * hg + (h + 1) * 64]
            bv_ext[0, h * 65 + 64] = 1.0
        in_maps.append({
            "xt": np.ascontiguousarray(x[b].T).astype(BF16),
            "wq": ext(Wq[:, sl] * SCALE).astype(BF16),
            "wk": ext(Wk[:, sl]).astype(BF16),
            "wv": wv_ext.astype(BF16),
            "wo": Wo[sl, :].astype(BF16),
            "bq": np.ascontiguousarray(bias_cols(ext(bq[sl] * SCALE)), np.float32),
            "bk": np.ascontiguousarray(bias_cols(ext(bk[sl])), np.float32),
            "bv": bv_ext.astype(BF16),
        })
    return in_maps


def kernel(x, Wq, bq, Wk, bk, Wv, bv, Wo, bo):
    from concourse import bass_utils

    x = np.asarray(x, np.float32)
    Wq = np.asarray(Wq, np.float32); bq = np.asarray(bq, np.float32)
    Wk = np.asarray(Wk, np.float32); bk = np.asarray(bk, np.float32)
    Wv = np.asarray(Wv, np.float32); bv = np.asarray(bv, np.float32)
    Wo = np.asarray(Wo, np.float32); bo = np.asarray(bo, np.float32)

    nc = _get_program()
    in_maps = _make_in_maps(x, Wq, bq, Wk, bk, Wv, bv, Wo)
    _cache["in_maps"] = in_maps
    res = bass_utils.run_bass_kernel_spmd(nc, in_maps, core_ids=list(range(N_CORES)))
    _cache["last_results"] = res

    out = np.zeros((B, N, DIM), np.float32)
    for c in range(N_CORES):
        out[c // 4] += res.results[c]["out"]
    out += bo[None, None, :]
    return out
